# revision 1
# baseline (speedup 1.0000x reference)
"""nn_ApproximateEuclideanAttention — 8-core Trainium2 Bass kernel.

Sharding (per spec hint): data-parallel over batch (2) x tensor-parallel over
heads (16 -> 4 groups of 4), one shard per NeuronCore. Each core computes its
head-group's attention and the partial output projection H @ Wo[:,sl].T; the
host sums the 4 partials per batch (+bo) and casts to fp32.

Device dataflow (validated numerically in numpy first; mean rel err 5.8e-3
vs fp32 reference with bf16 rounding everywhere, budget 2e-2):
  - activations "T-land" (embed on partitions): K^T/Q^T = W^T.T @ x^T; V in
    N-land (seq on partitions) for the Z-reduction.
  - The per-row ||Q||^2 and all ||L||^2 terms cancel in out/norm, so
    Phi_Q' = exp(2 L Q^T / tau), Phi_K' = exp((2 K L^T - ||K||^2)/tau), and
    the 64x64 solve matrix is Wexp = exp(2 L L^T / tau):
      out = (Phi_Q' A Z') / (Phi_Q' A s'),  A = Wexp^{-1}
  - ||K||^2 folded into the E_K matmul as a second accumulating matmul with
    a const blockdiag(-0.5) moving operand; Phi via one Exp activation.
  - A via 20 fp32 Newton-Schulz iters (X0 = I/256); W/X/G/P kept as
    per-pair blockdiag (128,128) tiles so every product is a plain
    full-128-contraction matmul (blockdiag x blockdiag stays blockdiag).
  - norm reciprocal via the fast custom-DVE approx; 1/norm broadcast over
    each head's 64 rows by a one-hot selector matmul on the PE; division
    fused into the H^T PSUM->SBUF eviction (tensor_mul).
  - PSUM discipline: start=True clears has_written for the whole bank, so
    banks with multiple interleaved accumulation ranges get exactly one
    start (first ranges overwrite unmarked elements, then accumulate).
"""

import numpy as np
import ml_dtypes

EMBED_DIM = 1024
NUM_HEADS = 16
HEAD_DIM = 64
NL = 64          # landmarks
N_CORES = 8
GROUPS = 4       # head groups -> 4 heads / 256 cols per core
C = 256          # local channel cols per core
N = 8192         # sequence length per batch
SLAB = 512
NSLAB = N // SLAB          # 16
NCH = SLAB // 128          # chunks per slab = 4
NS_ITERS = 20
DEBUG_DUMP = False

BF16 = ml_dtypes.bfloat16

LAST_RESULTS = None  # BassKernelResults of the most recent device run


# ---------------------------------------------------------------------------
# device program
# ---------------------------------------------------------------------------

def _build_bass(tau: float):
    import concourse.bass as bass
    import concourse.tile as tile
    from concourse import bacc, mybir


    f32 = mybir.dt.float32
    bf = mybir.dt.bfloat16
    Exp = mybir.ActivationFunctionType.Exp
    Square = mybir.ActivationFunctionType.Square
    Copy = mybir.ActivationFunctionType.Copy

    nc = bacc.Bacc("TRN2", target_bir_lowering=False, debug=False,
                   num_devices=N_CORES)

    xT_d = nc.dram_tensor("xT", [EMBED_DIM, N], bf, kind="ExternalInput")
    xlT_d = nc.dram_tensor("xlT", [EMBED_DIM, NL], bf, kind="ExternalInput")
    wkT_d = nc.dram_tensor("wkT", [EMBED_DIM, C], bf, kind="ExternalInput")
    wqT_d = nc.dram_tensor("wqT", [EMBED_DIM, C], bf, kind="ExternalInput")
    wvT_d = nc.dram_tensor("wvT", [EMBED_DIM, C], bf, kind="ExternalInput")
    woT_d = nc.dram_tensor("woT", [C, EMBED_DIM], bf, kind="ExternalInput")
    sqsel_d = nc.dram_tensor("sqsel", [128, 128], bf, kind="ExternalInput")
    id2w_d = nc.dram_tensor("id2w", [128, 128], f32, kind="ExternalInput")
    ones_d = nc.dram_tensor("onesc", [128, 1], bf, kind="ExternalInput")
    bsel_d = nc.dram_tensor("bsel", [4, 2, 128], bf, kind="ExternalInput")
    out_d = nc.dram_tensor("opart", [N, EMBED_DIM], bf, kind="ExternalOutput")

    # dram views
    xT_r = xT_d.ap().rearrange("(ci p) n -> p ci n", p=128)       # (128,8,N)
    xlT_r = xlT_d.ap().rearrange("(ci p) l -> p ci l", p=128)     # (128,8,64)
    wk_r = wkT_d.ap().rearrange("(ci p) e -> p ci e", p=128)      # (128,8,256)
    wq_r = wqT_d.ap().rearrange("(ci p) e -> p ci e", p=128)
    wv_r = wvT_d.ap().rearrange("(ci p) e -> p ci e", p=128)
    wo_r = woT_d.ap().rearrange("(ct p) e -> p ct e", p=128)      # (128,2,1024)
    out_r = out_d.ap().rearrange("(s c p) e -> s p c e", p=128, c=NCH)

    sc = 2.0 / tau

    with tile.TileContext(nc) as tc:
        import contextlib
        ctx = contextlib.ExitStack()
        with ctx:
            singles = ctx.enter_context(tc.tile_pool(name="singles", bufs=1))
            big = ctx.enter_context(tc.tile_pool(name="big", bufs=1))
            slabs = ctx.enter_context(tc.tile_pool(name="slabs", bufs=2))
            nspool = ctx.enter_context(tc.tile_pool(name="nspool", bufs=2))
            ps_big = ctx.enter_context(
                tc.tile_pool(name="ps_big", bufs=2, space="PSUM"))
            ps_v = ctx.enter_context(
                tc.tile_pool(name="ps_v", bufs=2, space="PSUM"))
            ps_e = ctx.enter_context(
                tc.tile_pool(name="ps_e", bufs=2, space="PSUM"))
            ps_ns = ctx.enter_context(
                tc.tile_pool(name="ps_ns", bufs=1, space="PSUM"))
            ps_z = ctx.enter_context(
                tc.tile_pool(name="ps_z", bufs=1, space="PSUM"))

            # ---- stage 0: weights + consts -------------------------------
            wk_sb = singles.tile([128, 8, C], bf)
            wq_sb = singles.tile([128, 8, C], bf)
            wv_sb = singles.tile([128, 8, C], bf)
            wo_sb = singles.tile([128, 2, EMBED_DIM], bf)
            sqsel_sb = singles.tile([128, 128], bf)
            id2w_sb = singles.tile([128, 128], f32)
            ones_sb = singles.tile([128, 1], bf)
            bsel_sb = singles.tile([4, 2, 128], bf)
            xlT_sb = singles.tile([128, 8, NL], bf)
            nc.sync.dma_start(out=wk_sb[:], in_=wk_r)
            xts_pref = {}
            for s in range(2):
                xpf = slabs.tile([128, 8, SLAB], bf, tag="xts", bufs=3,
                                 name=f"xtspref{s}")
                nc.sync.dma_start(out=xpf[:],
                                  in_=xT_r[:, :, s * SLAB:(s + 1) * SLAB])
                xts_pref[s] = xpf
            nc.sync.dma_start(out=wq_sb[:], in_=wq_r)
            nc.sync.dma_start(out=wv_sb[:], in_=wv_r)
            nc.sync.dma_start(out=wo_sb[:], in_=wo_r)
            nc.sync.dma_start(out=sqsel_sb[:], in_=sqsel_d.ap())
            nc.sync.dma_start(out=id2w_sb[:], in_=id2w_d.ap())
            nc.sync.dma_start(out=ones_sb[:], in_=ones_d.ap())
            nc.sync.dma_start(out=bsel_sb[:], in_=bsel_d.ap())
            nc.sync.dma_start(out=xlT_sb[:], in_=xlT_r)

            # warmup: absorb the const-bias-AP DMA wait into one tiny ACT op
            # (walrus allows only a couple of sync waits per instruction)
            warm = singles.tile([1, 1], f32)
            nc.scalar.activation(warm[:], id2w_sb[0:1, 0:1], Exp)

            # ---- stage 1: landmarks L^T (256, 64) ------------------------
            LT_sb = singles.tile([128, 2, NL], bf)
            for co in range(2):
                L_ps = ps_big.tile([128, NL], f32, tag="psb")
                for ci in range(8):
                    nc.tensor.matmul(
                        L_ps[:], wk_sb[:, ci, co * 128:(co + 1) * 128],
                        xlT_sb[:, ci, :], start=(ci == 0), stop=(ci == 7))
                nc.vector.tensor_copy(LT_sb[:, co, :], L_ps[:])

            # blockdiag(L^T) per pair (for S_Q lhsT and E_K rhs)
            bdl = singles.tile([128, 2, 128], bf)
            nc.vector.memset(bdl[:], 0.0)
            for t in range(2):
                nc.vector.tensor_copy(bdl[0:64, t, 0:64], LT_sb[0:64, t, :])
                nc.vector.tensor_copy(bdl[64:128, t, 64:128], LT_sb[64:128, t, :])

            # ---- stage 2: Wexp + Newton-Schulz inverse -------------------
            # Everything blockdiag per pair: blockdiag x blockdiag stays
            # blockdiag through plain full-128-contraction matmuls, so no
            # tile_position and no per-iter repacks are needed.
            W_ps = ps_big.tile([128, 128], f32, tag="psb")
            for t in range(2):
                nc.tensor.matmul(W_ps[:, 64 * t:64 * t + 64],
                                 bdl[:, t, :], LT_sb[:, t, :])
            Wf_sb = singles.tile([128, 128], f32)
            nc.scalar.activation(Wf_sb[:], W_ps[:], Exp, scale=sc)
            W_bd = [singles.tile([128, 128], f32, tag=f"wbd{t}", name=f"wbd{t}")
                    for t in range(2)]
            for t in range(2):
                nc.vector.memset(W_bd[t][:], 0.0)
                nc.vector.tensor_copy(W_bd[t][0:64, 0:64],
                                      Wf_sb[0:64, 64 * t:64 * t + 64])
                nc.vector.tensor_copy(W_bd[t][64:128, 64:128],
                                      Wf_sb[64:128, 64 * t:64 * t + 64])

            X_ping = [singles.tile([128, 128], f32, tag=f"xa{t}", name=f"xa{t}")
                      for t in range(2)]
            X_pong = [singles.tile([128, 128], f32, tag=f"xb{t}", name=f"xb{t}")
                      for t in range(2)]
            for t in range(2):
                nc.vector.tensor_scalar_mul(X_ping[t][:], id2w_sb[:],
                                            1.0 / 512.0)
            cur, nxt = X_ping, X_pong
            for it in range(NS_ITERS):
                for t in range(2):
                    P_ps = ps_ns.tile([128, 128], f32, tag="nsp")
                    nc.tensor.matmul(P_ps[:], W_bd[t][:], cur[t][:])
                    G_sb = nspool.tile([128, 128], f32, tag="nsg")
                    nc.vector.tensor_sub(G_sb[:], id2w_sb[:], P_ps[:])
                    Xp_ps = ps_ns.tile([128, 128], f32, tag="nsp")
                    nc.tensor.matmul(Xp_ps[:], cur[t][:], G_sb[:])
                    nc.vector.tensor_copy(nxt[t][:], Xp_ps[:])
                cur, nxt = nxt, cur
            M_bd = cur  # fp32 blockdiag inverse per pair

            # ---- stage 3: streaming projections + Phi + Z ----------------
            phiQ_sb = big.tile([128, 2, N], bf)
            Zacc_a = singles.tile([128, 258], f32)
            Zacc_b = singles.tile([128, 258], f32)
            for s in range(NSLAB):
                nsl = slice(s * SLAB, (s + 1) * SLAB)
                if s in xts_pref:
                    xts = xts_pref.pop(s)
                else:
                    xts = slabs.tile([128, 8, SLAB], bf, tag="xts", bufs=3)
                    nc.sync.dma_start(out=xts[:], in_=xT_r[:, :, nsl])

                # K^T and squares
                KT = slabs.tile([128, 2, SLAB], bf, tag="kt", bufs=3)
                sqKT = slabs.tile([128, 2, SLAB], bf, tag="sqkt", bufs=3)
                for co in range(2):
                    K_ps = ps_big.tile([128, SLAB], f32, tag="psb")
                    for ci in range(8):
                        nc.tensor.matmul(
                            K_ps[:], wk_sb[:, ci, co * 128:(co + 1) * 128],
                            xts[:, ci, :], start=(ci == 0), stop=(ci == 7))
                    nc.vector.tensor_copy(KT[:, co, :], K_ps[:])
                    nc.scalar.activation(sqKT[:, co, :], K_ps[:], Square)

                # Q^T -> Phi_Q'
                QT = slabs.tile([128, 2, SLAB], bf, tag="qt", bufs=3)
                for co in range(2):
                    Q_ps = ps_big.tile([128, SLAB], f32, tag="psb")
                    for ci in range(8):
                        nc.tensor.matmul(
                            Q_ps[:], wq_sb[:, ci, co * 128:(co + 1) * 128],
                            xts[:, ci, :], start=(ci == 0), stop=(ci == 7))
                    nc.vector.tensor_copy(QT[:, co, :], Q_ps[:])
                for t in range(2):
                    SQ_ps = ps_big.tile([128, SLAB], f32, tag="psb")
                    nc.tensor.matmul(SQ_ps[:], bdl[:, t, :], QT[:, t, :])
                    nc.scalar.activation(phiQ_sb[:, t, nsl], SQ_ps[:], Exp,
                                         scale=sc)

                # V (N-land), half-slab psums; Vb laid out per (chunk,
                # pair) as [V(128) | ones(1)] so Z and its row-sum come from
                # one N=129 matmul.
                Vb = slabs.tile([128, NCH, 2 * 129], bf, tag="vb", bufs=3)
                nc.vector.memset(Vb[:, :, 128:129], 1.0)
                nc.vector.memset(Vb[:, :, 257:258], 1.0)
                for hf in range(2):
                    V_ps = ps_v.tile([128, 2 * C], f32, tag="psv")
                    for c in (2 * hf, 2 * hf + 1):
                        for ci in range(8):
                            nc.tensor.matmul(
                                V_ps[:, (c % 2) * C:(c % 2 + 1) * C],
                                xts[:, ci, c * 128:(c + 1) * 128],
                                wv_sb[:, ci, :], start=(ci == 0),
                                stop=(ci == 7))
                    for c in (2 * hf, 2 * hf + 1):
                        for t in range(2):
                            nc.vector.tensor_copy(
                                Vb[:, c, 129 * t:129 * t + 128],
                                V_ps[:, (c % 2) * C + 128 * t:
                                     (c % 2) * C + 128 * t + 128])

                # E_K (N-land) -> Phi_K', half-slab psums
                phiK = slabs.tile([128, NCH, C], bf, tag="phik", bufs=3)
                for hf in range(2):
                    E_ps = ps_e.tile([128, 2 * C], f32, tag="pse")
                    for c in (2 * hf, 2 * hf + 1):
                        for t in range(2):
                            cs = slice((c % 2) * C + 128 * t,
                                       (c % 2) * C + 128 * t + 128)
                            nc.tensor.matmul(E_ps[:, cs],
                                             KT[:, t, c * 128:(c + 1) * 128],
                                             bdl[:, t, :], start=True,
                                             stop=False)
                            nc.tensor.matmul(E_ps[:, cs],
                                             sqKT[:, t, c * 128:(c + 1) * 128],
                                             sqsel_sb[:], start=False,
                                             stop=True)
                    nc.scalar.activation(phiK[:, 2 * hf:2 * hf + 2, :],
                                         E_ps[:], Exp, scale=sc)

                # Z for this slab (short PSUM accumulation groups), then
                # accumulated across slabs in SBUF (ping-pong DVE adds)
                # NOTE: start=True clears has_written for the WHOLE PSUM
                # bank, so emit exactly one start per bank: later ranges'
                # first writes hit unmarked elements and overwrite; their
                # subsequent writes accumulate.
                Z_ps = ps_z.tile([128, 258], f32, tag="zslab")
                for c in range(NCH):
                    for t in range(2):
                        zc = 129 * t
                        nc.tensor.matmul(
                            Z_ps[:, zc:zc + 129],
                            phiK[:, c, 128 * t:128 * t + 128],
                            Vb[:, c, 129 * t:129 * t + 129],
                            start=(c == 0 and t == 0),
                            stop=(c == NCH - 1 and t == 1),
                            skip_group_check=True)
                if s == 0:
                    nc.vector.tensor_copy(Zacc_a[:], Z_ps[:])
                elif s % 2 == 1:
                    nc.vector.tensor_add(Zacc_b[:], Z_ps[:], Zacc_a[:])
                else:
                    nc.vector.tensor_add(Zacc_a[:], Z_ps[:], Zacc_b[:])

            # ---- stage 4: solve application + norm -----------------------
            Zs_sb = Zacc_b if NSLAB % 2 == 0 else Zacc_a
            # pack valid Z blocks so blockdiag(M) applies in one matmul/pair:
            # Zp cols [65t:65t+64] rows 0:64 <- Z0, rows 64:128 <- Z1; col
            # 65t+64 <- s (both halves valid).
            Zp_sb = singles.tile([128, 130], f32)
            for t in range(2):
                nc.vector.tensor_copy(Zp_sb[0:64, 65 * t:65 * t + 64],
                                      Zs_sb[0:64, 129 * t:129 * t + 64])
                nc.vector.tensor_copy(Zp_sb[64:128, 65 * t:65 * t + 64],
                                      Zs_sb[64:128, 129 * t + 64:129 * t + 128])
                nc.vector.tensor_copy(Zp_sb[:, 65 * t + 64:65 * t + 65],
                                      Zs_sb[:, 129 * t + 128:129 * t + 129])
            Y_ps = ps_big.tile([128, 130], f32, tag="psb")
            for t in range(2):
                nc.tensor.matmul(Y_ps[:, 65 * t:65 * t + 65],
                                 M_bd[t][:], Zp_sb[:, 65 * t:65 * t + 65])
            Yb_sb = singles.tile([128, 130], bf)
            nc.vector.tensor_copy(Yb_sb[:], Y_ps[:])
            # blockdiag(Y) per pair for the H matmuls
            Y_bd = singles.tile([128, 2, 128], bf)
            nc.vector.memset(Y_bd[:], 0.0)
            for t in range(2):
                nc.vector.tensor_copy(Y_bd[0:64, t, 0:64],
                                      Yb_sb[0:64, 65 * t:65 * t + 64])
                nc.vector.tensor_copy(Y_bd[64:128, t, 64:128],
                                      Yb_sb[64:128, 65 * t:65 * t + 64])

            selY = []
            for t in range(2):
                sl_t = singles.tile([128, 4], bf, tag=f"sely{t}", name=f"sely{t}")
                nc.vector.memset(sl_t[:], 0.0)
                for hh in range(2):
                    r = slice(64 * hh, 64 * hh + 64)
                    nc.vector.tensor_copy(
                        sl_t[r, 2 * t + hh:2 * t + hh + 1],
                        Yb_sb[r, 65 * t + 64:65 * t + 65])
                selY.append(sl_t)

            rnorm_f = big.tile([4, N], f32)
            # ---- stage 5: per-slab norm chain, H^T, divide, O-proj -------
            # broadcast 1/norm over each head's 64 rows with a one-hot
            # selector matmul (PE), avoiding any DRAM bounce.
            for s in range(NSLAB):
                nsl = slice(s * SLAB, (s + 1) * SLAB)
                n_ps = ps_ns.tile([4, SLAB], f32, tag="nsp")
                for t in range(2):
                    nc.tensor.matmul(n_ps[:], selY[t][:],
                                     phiQ_sb[:, t, nsl],
                                     start=(t == 0), stop=(t == 1))
                nc.vector.reciprocal_approx_fast(out=rnorm_f[:, nsl],
                                                 in_=n_ps[:])
                rnb = slabs.tile([4, SLAB], bf, tag="rnb", bufs=3)
                nc.vector.tensor_copy(rnb[:], rnorm_f[:, nsl])
                rnE = slabs.tile([128, 2, SLAB], bf, tag="rne", bufs=3)
                for t in range(2):
                    rn_ps = ps_z.tile([128, SLAB], f32, tag="zslab",
                                      name=f"rnps{s}_{t}")
                    nc.tensor.matmul(rn_ps[:], bsel_sb[:, t, :], rnb[:])
                    nc.scalar.activation(rnE[:, t, :], rn_ps[:], Copy)

                HT = slabs.tile([128, 2, SLAB], bf, tag="ht", bufs=3)
                for t in range(2):
                    H_ps = ps_big.tile([128, SLAB], f32, tag="psb")
                    nc.tensor.matmul(H_ps[:], Y_bd[:, t, :],
                                     phiQ_sb[:, t, nsl])
                    nc.vector.tensor_mul(HT[:, t, :], H_ps[:], rnE[:, t, :])
                oout = slabs.tile([128, NCH, EMBED_DIM], bf, tag="oout",
                                  bufs=3)
                for c in range(NCH):
                    # ct outer / eh inner: consecutive matmuls share the same
                    # stationary operand (HT chunk), halving weight loads
                    O_pair = [
                        ps_v.tile([128, 512], f32, tag="psv",
                                  name=f"ops{s}_{c}_0"),
                        ps_e.tile([128, 512], f32, tag="pse",
                                  name=f"ops{s}_{c}_1"),
                    ]
                    for ct in range(2):
                        for eh in range(2):
                            nc.tensor.matmul(
                                O_pair[eh][:],
                                HT[:, ct, c * 128:(c + 1) * 128],
                                wo_sb[:, ct, eh * 512:eh * 512 + 512],
                                start=(ct == 0), stop=(ct == 1))
                    nc.vector.tensor_copy(oout[:, c, 0:512], O_pair[0][:])
                    nc.scalar.activation(oout[:, c, 512:1024], O_pair[1][:],
                                         Copy)
                nc.sync.dma_start(out=out_r[s], in_=oout[:])
            if DEBUG_DUMP:
                zdbg = nc.dram_tensor("zdbg", [128, 258], f32,
                                      kind="ExternalOutput")
                ydbg = nc.dram_tensor("ydbg", [128, 130], f32,
                                      kind="ExternalOutput")
                mdbg = nc.dram_tensor("mdbg", [128, 256], f32,
                                      kind="ExternalOutput")
                rdbg = nc.dram_tensor("rdbg", [4, N], f32,
                                      kind="ExternalOutput")
                ybf = singles.tile([128, 130], f32)
                nc.vector.tensor_copy(ybf[:], Yb_sb[:])
                nc.sync.dma_start(out=zdbg.ap(), in_=Zs_sb[:])
                nc.sync.dma_start(out=ydbg.ap(), in_=ybf[:])
                for t in range(2):
                    nc.sync.dma_start(out=mdbg.ap()[:, 128*t:128*t+128],
                                      in_=M_bd[t][:])
                nc.sync.dma_start(out=rdbg.ap(), in_=rnorm_f[:])
    nc.compile()
    return nc


_NC_CACHE = None


def _get_nc(tau):
    global _NC_CACHE
    if _NC_CACHE is None:
        _NC_CACHE = _build_bass(tau)
    return _NC_CACHE


# ---------------------------------------------------------------------------
# host marshalling
# ---------------------------------------------------------------------------

def _consts():
    sqsel = np.zeros((128, 128), np.float32)
    sqsel[0:64, 0:64] = -0.5
    sqsel[64:128, 64:128] = -0.5
    id2w = 2.0 * np.eye(128, dtype=np.float32)
    onesc = np.ones((128, 1), np.float32)
    bsel = np.zeros((4, 2, 128), np.float32)
    p = np.arange(128)
    for t in range(2):
        bsel[:, t, :][2 * t + p // 64, p] = 1.0
    return (sqsel.astype(BF16), id2w, onesc.astype(BF16), bsel.astype(BF16))


def _kernel_device(query, Wq, Wk, Wv, Wo, bo, tau, idx):
    global LAST_RESULTS
    from concourse.bass_utils import run_bass_kernel_spmd

    nc = _get_nc(tau)
    b, n, _ = query.shape

    sqsel, id2w, onesc, bsel = _consts()
    WkT = np.ascontiguousarray(Wk.T).astype(BF16)
    WqT = np.ascontiguousarray(Wq.T).astype(BF16)
    WvT = np.ascontiguousarray(Wv.T).astype(BF16)
    WoT = np.ascontiguousarray(Wo.T).astype(BF16)

    in_maps = []
    for bi in range(b):
        xT = np.ascontiguousarray(query[bi].T).astype(BF16)
        xlT = np.ascontiguousarray(query[bi][idx].T).astype(BF16)
        for g in range(GROUPS):
            sl = slice(g * C, (g + 1) * C)
            in_maps.append({
                "xT": xT,
                "xlT": xlT,
                "wkT": np.ascontiguousarray(WkT[:, sl]),
                "wqT": np.ascontiguousarray(WqT[:, sl]),
                "wvT": np.ascontiguousarray(WvT[:, sl]),
                "woT": np.ascontiguousarray(WoT[sl, :]),
                "sqsel": sqsel,
                "id2w": id2w,
                "onesc": onesc,
                "bsel": bsel,
            })

    res = run_bass_kernel_spmd(nc, in_maps, core_ids=list(range(N_CORES)))
    LAST_RESULTS = res

    out = np.zeros((b, n, EMBED_DIM), np.float32)
    for bi in range(b):
        for g in range(GROUPS):
            out[bi] += res.results[bi * GROUPS + g]["opart"].astype(np.float32)
    out += bo
    return out


def _kernel_numpy(query, Wq, bq, Wk, bk, Wv, bv, Wo, bo, tau, idx):
    """Reference-faithful fallback (nonzero biases etc.)."""
    b, n, _ = query.shape
    out = np.zeros((b, n, EMBED_DIM), np.float32)
    for bi in range(b):
        x = query[bi]
        Q = (x @ Wq.T + bq).reshape(n, NUM_HEADS, HEAD_DIM).transpose(1, 0, 2)
        K = (x @ Wk.T + bk).reshape(n, NUM_HEADS, HEAD_DIM).transpose(1, 0, 2)
        V = (x @ Wv.T + bv).reshape(n, NUM_HEADS, HEAD_DIM).transpose(1, 0, 2)
        L = K[:, idx, :]
        def sqd(X, Lm):
            Xn = np.sum(X * X, -1, keepdims=True)
            Ln = np.sum(Lm * Lm, -1, keepdims=True)
            return np.maximum(Xn + np.swapaxes(Ln, -2, -1)
                              - 2.0 * np.einsum("hnd,hkd->hnk", X, Lm), 0.0)
        PhiQ = np.exp(-sqd(Q, L) / tau)
        PhiK = np.exp(-sqd(K, L) / tau)
        Wk_ = np.exp(-sqd(L, L) / tau) + 1e-6 * np.eye(NL, dtype=np.float32)
        Z = np.einsum("hnk,hnd->hkd", PhiK, V)
        Y = np.linalg.solve(Wk_, Z)
        ou = np.einsum("hnk,hkd->hnd", PhiQ, Y)
        sY = np.linalg.solve(Wk_, PhiK.sum(1)[..., None])
        nrm = np.maximum(np.einsum("hnk,hko->hno", PhiQ, sY), 1e-10)
        H = (ou / nrm).transpose(1, 0, 2).reshape(n, EMBED_DIM)
        out[bi] = H @ Wo.T
    return out + bo


def kernel(query, Wq, bq, Wk, bk, Wv, bv, Wo, bo, temperature, landmark_idx):
    query = np.asarray(query, dtype=np.float32)
    Wq, Wk, Wv, Wo = (np.asarray(w, np.float32) for w in (Wq, Wk, Wv, Wo))
    bq, bk, bv, bo = (np.asarray(x, np.float32) for x in (bq, bk, bv, bo))
    tau = float(np.asarray(temperature))
    idx = np.asarray(landmark_idx).astype(np.int64)

    if (query.shape != (2, N, EMBED_DIM) or idx.shape != (NL,)
            or np.any(bq) or np.any(bk) or np.any(bv)):
        return _kernel_numpy(query, Wq, bq, Wk, bk, Wv, bv, Wo, bo, tau, idx)
    return _kernel_device(query, Wq, Wk, Wv, Wo, bo, tau, idx).astype(
        np.float32, copy=False)



# revision 9
# speedup vs baseline: 1.1092x; 1.1092x over previous
"""nn_ApproximateEuclideanAttention — 8-core Trainium2 Bass kernel (v2).

Sharding: data-parallel over batch (2) x tensor-parallel over heads (16 -> 4
groups of 4), one shard per NeuronCore. Each core computes its head-group's
attention and the partial output projection (written TRANSPOSED, embed-major);
the host sums the 4 partials per batch (+bo).

v2 restructure (validated numerically in val_v2.py, mean rel err 5.6e-3):
  - Q projection folded into the landmarks: S_Q = A_q x^T with
    A_q = blockdiag(L) Wq_pair, so phiQ comes from one accumulating matmul
    chain per pair (no QT eviction, no separate S_Q matmul).
  - Wo folded into Y: YT = Zp^T A (one matmul per pair, A symmetric), then
    Ypack = blockdiag(Y^T)^T wo_pair once; the per-slab H matmul disappears
    and the O-projection is outT_e += Ypack_e^T (phiQ * 1/norm).
  - norm broadcast: the norm matmul uses a blockdiag stationary selYE whose
    columns repeat sY per 64-row head block, directly producing the
    (128,seq) broadcast layout; ACT Reciprocal gives 1/norm in bf16. The
    old bsel broadcast matmuls + evictions disappear.
  - Z accumulates across all 16 slabs in one persistent PSUM bank (single
    start/stop pair; start=True clears has_written for the whole bank so
    only the very first matmul may carry it).
  - Startup: initial DMAs issued in parallel on sync+scalar+gpsimd queues,
    xlT early; junk warmup matmuls keep/get the PE HAM warm during the
    initial DMA wait.
  - Output DMA chunked (2 per slab) to shrink the end-of-kernel tail.
"""

import numpy as np
import ml_dtypes

EMBED_DIM = 1024
NUM_HEADS = 16
HEAD_DIM = 64
NL = 64          # landmarks
N_CORES = 8
GROUPS = 4       # head groups -> 4 heads / 256 cols per core
C = 256          # local channel cols per core
N = 8192         # sequence length per batch
SLAB = 512
NSLAB = N // SLAB          # 16
NCH = SLAB // 128          # chunks per slab = 4
NS_ITERS = 20
DEBUG_DUMP = False

BF16 = ml_dtypes.bfloat16

LAST_RESULTS = None  # BassKernelResults of the most recent device run


# ---------------------------------------------------------------------------
# device program
# ---------------------------------------------------------------------------

def _build_bass(tau: float):
    import concourse.bass as bass
    import concourse.tile as tile
    from concourse import bacc, mybir

    f32 = mybir.dt.float32
    f32r = mybir.dt.float32r
    bf = mybir.dt.bfloat16
    Exp = mybir.ActivationFunctionType.Exp
    Square = mybir.ActivationFunctionType.Square
    Copy = mybir.ActivationFunctionType.Copy

    nc = bacc.Bacc("TRN2", target_bir_lowering=False, debug=False,
                   num_devices=N_CORES)

    xT_d = nc.dram_tensor("xT", [EMBED_DIM, N], bf, kind="ExternalInput")
    xlT_d = nc.dram_tensor("xlT", [EMBED_DIM, NL], bf, kind="ExternalInput")
    wkT_d = nc.dram_tensor("wkT", [EMBED_DIM, C], bf, kind="ExternalInput")
    wq2_d = nc.dram_tensor("wq2", [C, EMBED_DIM], bf, kind="ExternalInput")
    wvT_d = nc.dram_tensor("wvT", [EMBED_DIM, C], bf, kind="ExternalInput")
    woT_d = nc.dram_tensor("woT", [C, EMBED_DIM], bf, kind="ExternalInput")
    sqsel_d = nc.dram_tensor("sqsel", [128, 128], bf, kind="ExternalInput")
    id2w_d = nc.dram_tensor("id2w", [128, 128], f32, kind="ExternalInput")
    out_d = nc.dram_tensor("opart", [EMBED_DIM, N], bf, kind="ExternalOutput")

    # dram views
    xT_r = xT_d.ap().rearrange("(ci p) n -> p ci n", p=128)       # (128,8,N)
    xlT_r = xlT_d.ap().rearrange("(ci p) l -> p ci l", p=128)     # (128,8,64)
    wk_r = wkT_d.ap().rearrange("(ci p) e -> p ci e", p=128)      # (128,8,256)
    wq2_r = wq2_d.ap().rearrange("(ct p) e -> p ct e", p=128)     # (128,2,1024)
    wv_r = wvT_d.ap().rearrange("(ci p) e -> p ci e", p=128)
    wo_r = woT_d.ap().rearrange("(ct p) e -> p ct e", p=128)      # (128,2,1024)
    out_rT = out_d.ap().rearrange("(e p) n -> p e n", p=128)      # (128,8,N)

    sc = 2.0 / tau
    XTS_BUFS = 4

    with tile.TileContext(nc) as tc:
        import contextlib
        ctx = contextlib.ExitStack()
        with ctx:
            singles = ctx.enter_context(tc.tile_pool(name="singles", bufs=1))
            big = ctx.enter_context(tc.tile_pool(name="big", bufs=1))
            slabs = ctx.enter_context(tc.tile_pool(name="slabs", bufs=2))
            nspool = ctx.enter_context(tc.tile_pool(name="nspool", bufs=2))
            ps_big = ctx.enter_context(
                tc.tile_pool(name="ps_big", bufs=2, space="PSUM"))
            ps_v = ctx.enter_context(
                tc.tile_pool(name="ps_v", bufs=2, space="PSUM"))
            ps_e = ctx.enter_context(
                tc.tile_pool(name="ps_e", bufs=2, space="PSUM"))
            ps_ns = ctx.enter_context(
                tc.tile_pool(name="ps_ns", bufs=1, space="PSUM"))
            ps_z = ctx.enter_context(
                tc.tile_pool(name="ps_z", bufs=1, space="PSUM"))

            # ---- stage 0: weights + consts, spread across DMA queues -----
            wk_sb = singles.tile([128, 8, C], bf)
            wq2_sb = singles.tile([128, 2, EMBED_DIM], bf)
            wv_sb = singles.tile([128, 8, C], bf)
            wo_sb = singles.tile([128, 2, EMBED_DIM], bf)
            sqsel_sb = singles.tile([128, 128], bf)
            id2w_sb = singles.tile([128, 128], f32)
            xlT_sb = singles.tile([128, 8, NL], bf)

            # PE warmup: junk matmuls with no DMA deps; run during the
            # initial DMA wait so HAM reaches K=8/8 before real work.
            junk = singles.tile([128, 256], bf)
            nc.vector.memset(junk[:], 0.0)
            warm_ps = ps_ns.tile([128, 256], f32, tag="nsp", name="warmps")
            for i in range(16):
                nc.tensor.matmul(warm_ps[:], junk[:, 0:128], junk[:],
                                 start=(i == 0), stop=(i == 15))

            nc.sync.dma_start(out=wk_sb[:], in_=wk_r)
            nc.scalar.dma_start(out=xlT_sb[:], in_=xlT_r)
            xts_pref = {}
            for s in range(XTS_BUFS):
                xpf = slabs.tile([128, 8, SLAB], bf, tag="xts", bufs=XTS_BUFS,
                                 name=f"xtspref{s}")
                nc.sync.dma_start(out=xpf[:],
                                  in_=xT_r[:, :, s * SLAB:(s + 1) * SLAB])
                xts_pref[s] = xpf
            nc.scalar.dma_start(out=wq2_sb[:], in_=wq2_r)
            nc.scalar.dma_start(out=wv_sb[:], in_=wv_r)
            nc.scalar.dma_start(out=wo_sb[:], in_=wo_r)
            nc.gpsimd.dma_start(out=sqsel_sb[:], in_=sqsel_d.ap())
            nc.gpsimd.dma_start(out=id2w_sb[:], in_=id2w_d.ap())

            # absorb the const-bias-AP DMA wait into one tiny ACT op
            warm1 = singles.tile([1, 1], f32)
            nc.scalar.activation(warm1[:], id2w_sb[0:1, 0:1], Exp)

            # ---- stage 1: landmarks L^T (256, 64), chan on partitions ----
            LT_sb = singles.tile([128, 2, NL], bf)
            for co in range(2):
                L_ps = ps_big.tile([128, NL], f32, tag="psb")
                for ci in range(8):
                    nc.tensor.matmul(
                        L_ps[:], wk_sb[:, ci, co * 128:(co + 1) * 128],
                        xlT_sb[:, ci, :], start=(ci == 0), stop=(ci == 7))
                nc.vector.tensor_copy(LT_sb[:, co, :], L_ps[:])

            # blockdiag(L^T) per pair (chan-part, land-cols)
            bdl = singles.tile([128, 2, 128], bf)
            nc.vector.memset(bdl[:], 0.0)
            for t in range(2):
                nc.vector.tensor_copy(bdl[0:64, t, 0:64], LT_sb[0:64, t, :])
                nc.vector.tensor_copy(bdl[64:128, t, 64:128], LT_sb[64:128, t, :])

            # ---- stage 1b: A_qT = wq2_pair^T-contract @ bdl  (Q folding) --
            A_qT = singles.tile([128, 8, 2, 128], bf)
            for t in range(2):
                for ch in range(0, 8, 2):
                    Aq_ps = ps_big.tile([128, 256], f32, tag="psb",
                                        name=f"aqps{t}_{ch}")
                    for k in range(2):
                        ci = ch + k
                        nc.tensor.matmul(
                            Aq_ps[:, 128 * k:128 * k + 128],
                            wq2_sb[:, t, ci * 128:(ci + 1) * 128],
                            bdl[:, t, :], start=(k == 0), stop=(k == 1),
                            skip_group_check=True)
                    for k in range(2):
                        nc.vector.tensor_copy(
                            A_qT[:, ch + k, t, :],
                            Aq_ps[:, 128 * k:128 * k + 128])

            # ---- stage 2: Wexp + Newton-Schulz inverse (f32r matmuls) ----
            W_ps = ps_big.tile([128, 128], f32, tag="psb")
            for t in range(2):
                nc.tensor.matmul(W_ps[:, 64 * t:64 * t + 64],
                                 bdl[:, t, :], LT_sb[:, t, :])
            Wf_sb = singles.tile([128, 128], f32)
            nc.scalar.activation(Wf_sb[:], W_ps[:], Exp, scale=sc)
            W_bd = [singles.tile([128, 128], f32, tag=f"wbd{t}", name=f"wbd{t}")
                    for t in range(2)]
            for t in range(2):
                nc.vector.memset(W_bd[t][:], 0.0)
                nc.vector.tensor_copy(W_bd[t][0:64, 0:64],
                                      Wf_sb[0:64, 64 * t:64 * t + 64])
                nc.vector.tensor_copy(W_bd[t][64:128, 64:128],
                                      Wf_sb[64:128, 64 * t:64 * t + 64])

            X_ping = [singles.tile([128, 128], f32, tag=f"xa{t}", name=f"xa{t}")
                      for t in range(2)]
            X_pong = [singles.tile([128, 128], f32, tag=f"xb{t}", name=f"xb{t}")
                      for t in range(2)]
            for t in range(2):
                nc.vector.tensor_scalar_mul(X_ping[t][:], id2w_sb[:],
                                            1.0 / 512.0)
            cur, nxt = X_ping, X_pong
            for it in range(NS_ITERS):
                for t in range(2):
                    P_ps = ps_ns.tile([128, 128], f32, tag="nsp")
                    nc.tensor.matmul(P_ps[:], W_bd[t][:], cur[t][:])
                    G_sb = nspool.tile([128, 128], f32, tag="nsg")
                    nc.vector.tensor_sub(G_sb[:], id2w_sb[:], P_ps[:])
                    Xp_ps = ps_ns.tile([128, 128], f32, tag="nsp")
                    nc.tensor.matmul(Xp_ps[:], cur[t][:], G_sb[:])
                    nc.vector.tensor_copy(nxt[t][:], Xp_ps[:])
                cur, nxt = nxt, cur
            M_bd = cur  # fp32 blockdiag inverse per pair

            # ---- stage 3: streaming projections + Phi + Z ----------------
            phiQ_sb = big.tile([128, 2, N], bf)
            Z_ps = ps_z.tile([128, 258], f32, tag="zacc")  # persistent bank

            def emit_sq(s, xts):
                nsl = slice(s * SLAB, (s + 1) * SLAB)
                for t in range(2):
                    SQ_ps = ps_big.tile([128, SLAB], f32, tag="psb",
                                        name=f"sqps{s}_{t}")
                    for ci in range(8):
                        nc.tensor.matmul(SQ_ps[:], A_qT[:, ci, t, :],
                                         xts[:, ci, :], start=(ci == 0),
                                         stop=(ci == 7))
                    nc.scalar.activation(phiQ_sb[:, t, nsl], SQ_ps[:], Exp,
                                         scale=sc)

            xts_last = None
            for s in range(NSLAB):
                if s in xts_pref:
                    xts = xts_pref.pop(s)
                else:
                    xts = slabs.tile([128, 8, SLAB], bf, tag="xts",
                                     bufs=XTS_BUFS)
                    nc.sync.dma_start(out=xts[:],
                                      in_=xT_r[:, :, s * SLAB:(s + 1) * SLAB])

                # K^T and squares
                KT = slabs.tile([128, 2, SLAB], bf, tag="kt", bufs=3)
                sqKT = slabs.tile([128, 2, SLAB], bf, tag="sqkt", bufs=3)
                for co in range(2):
                    K_ps = ps_big.tile([128, SLAB], f32, tag="psb")
                    for ci in range(8):
                        nc.tensor.matmul(
                            K_ps[:], wk_sb[:, ci, co * 128:(co + 1) * 128],
                            xts[:, ci, :], start=(ci == 0), stop=(ci == 7))
                    nc.vector.tensor_copy(KT[:, co, :], K_ps[:])
                    nc.scalar.activation(sqKT[:, co, :], K_ps[:], Square)

                # phiQ (folded Q projection); slab 15 deferred to stage 4
                if s < NSLAB - 1:
                    emit_sq(s, xts)
                else:
                    xts_last = xts

                # V (N-land), half-slab psums; Vb laid out per (chunk,
                # pair) as [V(128) | ones(1)].
                Vb = slabs.tile([128, NCH, 2 * 129], bf, tag="vb", bufs=3)
                nc.vector.memset(Vb[:, :, 128:129], 1.0)
                nc.vector.memset(Vb[:, :, 257:258], 1.0)
                for hf in range(2):
                    V_ps = ps_v.tile([128, 2 * C], f32, tag="psv")
                    for c in (2 * hf, 2 * hf + 1):
                        for ci in range(8):
                            nc.tensor.matmul(
                                V_ps[:, (c % 2) * C:(c % 2 + 1) * C],
                                xts[:, ci, c * 128:(c + 1) * 128],
                                wv_sb[:, ci, :], start=(ci == 0),
                                stop=(ci == 7))
                    for c in (2 * hf, 2 * hf + 1):
                        for t in range(2):
                            nc.vector.tensor_copy(
                                Vb[:, c, 129 * t:129 * t + 128],
                                V_ps[:, (c % 2) * C + 128 * t:
                                     (c % 2) * C + 128 * t + 128])

                # E_K (N-land) -> Phi_K', half-slab psums
                phiK = slabs.tile([128, NCH, C], bf, tag="phik", bufs=3)
                for hf in range(2):
                    E_ps = ps_e.tile([128, 2 * C], f32, tag="pse")
                    for c in (2 * hf, 2 * hf + 1):
                        for t in range(2):
                            cs = slice((c % 2) * C + 128 * t,
                                       (c % 2) * C + 128 * t + 128)
                            nc.tensor.matmul(E_ps[:, cs],
                                             KT[:, t, c * 128:(c + 1) * 128],
                                             bdl[:, t, :], start=True,
                                             stop=False)
                            nc.tensor.matmul(E_ps[:, cs],
                                             sqKT[:, t, c * 128:(c + 1) * 128],
                                             sqsel_sb[:], start=False,
                                             stop=True)
                    nc.scalar.activation(phiK[:, 2 * hf:2 * hf + 2, :],
                                         E_ps[:], Exp, scale=sc)

                # Z accumulated across ALL slabs in the persistent PSUM
                # bank: exactly one start (s==0 first mm) / stop (last).
                for c in range(NCH):
                    for t in range(2):
                        zc = 129 * t
                        nc.tensor.matmul(
                            Z_ps[:, zc:zc + 129],
                            phiK[:, c, 128 * t:128 * t + 128],
                            Vb[:, c, 129 * t:129 * t + 129],
                            start=(s == 0 and c == 0 and t == 0),
                            stop=(s == NSLAB - 1 and c == NCH - 1 and t == 1),
                            skip_group_check=True)

            # ---- stage 4: solve application (transposed), Wo folding -----
            # Zp pack from the persistent Z bank: cols [65t:65t+64] rows
            # 0:64 <- Z0 diag-block, rows 64:128 <- Z1 diag-block; col
            # 65t+64 <- s' (both halves valid).
            Zp_sb = singles.tile([128, 130], f32)
            for t in range(2):
                nc.vector.tensor_copy(Zp_sb[0:64, 65 * t:65 * t + 64],
                                      Z_ps[0:64, 129 * t:129 * t + 64])
                nc.vector.tensor_copy(Zp_sb[64:128, 65 * t:65 * t + 64],
                                      Z_ps[64:128, 129 * t + 64:129 * t + 128])
                nc.vector.tensor_copy(Zp_sb[:, 65 * t + 64:65 * t + 65],
                                      Z_ps[:, 129 * t + 128:129 * t + 129])

            # YT_pair = Zp_slice^T @ M_bd  (65 part: 64 chan + sY row? no —
            # rows = 64 chan cols of Zp slice + the s' col -> row 64 unused
            # here; Y^T blocks live in rows 0:64, cols [land0|land1]).
            YT_ps = []
            for t in range(2):
                yt = ps_big.tile([128, 128], f32, tag="psb", name=f"ytps{t}")
                nc.tensor.matmul(yt[0:65, :],
                                 Zp_sb[:, 65 * t:65 * t + 65],
                                 M_bd[t][:])
                YT_ps.append(yt)

            # sY = A @ s' (orig orientation, partition-dim vector per pair)
            sY_ps = ps_ns.tile([128, 2], f32, tag="nsp", name="syps")
            for t in range(2):
                nc.tensor.matmul(
                    sY_ps[:, t:t + 1], M_bd[t][:],
                    Zp_sb[:, 65 * t + 64:65 * t + 65],
                    skip_group_check=True)

            # deferred slab-15 phiQ fills the PE while the DVE does the
            # stage-4 packing / selYE construction
            emit_sq(NSLAB - 1, xts_last)

            # Y_bdT (chan-part, land-cols) blockdiag, bf16
            Y_bdT = singles.tile([128, 2, 128], bf)
            nc.vector.memset(Y_bdT[:], 0.0)
            for t in range(2):
                nc.vector.tensor_copy(Y_bdT[0:64, t, 0:64],
                                      YT_ps[t][0:64, 0:64])
                nc.vector.tensor_copy(Y_bdT[64:128, t, 64:128],
                                      YT_ps[t][0:64, 64:128])

            # Ypack[t] = Y_bdT[t]^T-contract @ wo_pair  (land-part, embed)
            Ypack = singles.tile([128, 2, EMBED_DIM], bf)
            for t in range(2):
                for eh in range(2):
                    yp_ps = ps_ns.tile([128, 512], f32, tag="nsp",
                                       name=f"ypps{t}_{eh}")
                    nc.tensor.matmul(yp_ps[:], Y_bdT[:, t, :],
                                     wo_sb[:, t, eh * 512:eh * 512 + 512])
                    nc.vector.tensor_copy(
                        Ypack[:, t, eh * 512:eh * 512 + 512], yp_ps[:])

            # selYE[t]: blockdiag broadcast of sY per 64-row head block
            selYE = singles.tile([128, 2, 128], bf)
            nc.vector.memset(selYE[:], 0.0)
            for t in range(2):
                for hh in range(2):
                    r = slice(64 * hh, 64 * hh + 64)
                    nc.vector.tensor_copy(
                        selYE[r, t, 64 * hh:64 * hh + 64],
                        sY_ps[r, t:t + 1].broadcast_to([64, 64]))

            # ---- stage 5: norm -> 1/norm -> phiQn -> transposed O-proj ---
            for s in range(NSLAB):
                nsl = slice(s * SLAB, (s + 1) * SLAB)
                rnE = slabs.tile([128, 2, SLAB], f32, tag="rne", bufs=3)
                phiQn = slabs.tile([128, 2, SLAB], bf, tag="pqn", bufs=3)
                for t in range(2):
                    n_ps = ps_big.tile([128, SLAB], f32, tag="psb",
                                       name=f"nps{s}_{t}")
                    nc.tensor.matmul(n_ps[:], selYE[:, t, :],
                                     phiQ_sb[:, t, nsl])
                    nc.vector.reciprocal_approx_fast(out=rnE[:, t, :],
                                                     in_=n_ps[:])
                    nc.vector.tensor_mul(phiQn[:, t, :], phiQ_sb[:, t, nsl],
                                         rnE[:, t, :])

                oout = slabs.tile([128, 8, SLAB], bf, tag="oout", bufs=3)
                for e in range(8):
                    pool = ps_v if e % 2 == 0 else ps_e
                    O_ps = pool.tile([128, SLAB], f32,
                                     tag=("psv" if e % 2 == 0 else "pse"),
                                     name=f"ops{s}_{e}")
                    for t in range(2):
                        nc.tensor.matmul(
                            O_ps[:], Ypack[:, t, e * 128:(e + 1) * 128],
                            phiQn[:, t, :], start=(t == 0), stop=(t == 1))
                    if e < 2:
                        nc.vector.tensor_copy(oout[:, e, :], O_ps[:])
                    else:
                        nc.scalar.activation(oout[:, e, :], O_ps[:], Copy)
                for half in range(2):
                    nc.sync.dma_start(
                        out=out_rT[:, 4 * half:4 * half + 4, nsl],
                        in_=oout[:, 4 * half:4 * half + 4, :])
                if DEBUG_DUMP and s == 0:
                    rne_dbg = nc.dram_tensor("rnedbg", [128, 2, SLAB], f32,
                                             kind="ExternalOutput")
                    pqn_dbg = nc.dram_tensor("pqndbg", [128, 2, SLAB], bf,
                                             kind="ExternalOutput")
                    nc.sync.dma_start(out=rne_dbg.ap(), in_=rnE[:])
                    nc.sync.dma_start(out=pqn_dbg.ap(), in_=phiQn[:])

            if DEBUG_DUMP:
                z_dbg = nc.dram_tensor("zdbg", [128, 258], f32,
                                       kind="ExternalOutput")
                zc_sb = singles.tile([128, 258], f32)
                nc.vector.tensor_copy(zc_sb[:], Z_ps[:])
                nc.sync.dma_start(out=z_dbg.ap(), in_=zc_sb[:])
                zp_dbg = nc.dram_tensor("zpdbg", [128, 130], f32,
                                        kind="ExternalOutput")
                nc.sync.dma_start(out=zp_dbg.ap(), in_=Zp_sb[:])
                yt_dbg = nc.dram_tensor("ytdbg", [128, 2, 128], f32,
                                        kind="ExternalOutput")
                yt_sb = singles.tile([128, 2, 128], f32)
                for t in range(2):
                    nc.vector.tensor_copy(yt_sb[:, t, :], YT_ps[t][:])
                nc.sync.dma_start(out=yt_dbg.ap(), in_=yt_sb[:])
                sy_dbg = nc.dram_tensor("sydbg", [128, 2], f32,
                                        kind="ExternalOutput")
                sy_sb = singles.tile([128, 2], f32)
                nc.vector.tensor_copy(sy_sb[:], sY_ps[:])
                nc.sync.dma_start(out=sy_dbg.ap(), in_=sy_sb[:])
                sel_dbg = nc.dram_tensor("seldbg", [128, 2, 128], bf,
                                         kind="ExternalOutput")
                nc.sync.dma_start(out=sel_dbg.ap(), in_=selYE[:])
                yp_dbg = nc.dram_tensor("ypdbg", [128, 2, 1024], bf,
                                        kind="ExternalOutput")
                nc.sync.dma_start(out=yp_dbg.ap(), in_=Ypack[:])
                ybdt_dbg = nc.dram_tensor("ybdtdbg", [128, 2, 128], bf,
                                          kind="ExternalOutput")
                nc.sync.dma_start(out=ybdt_dbg.ap(), in_=Y_bdT[:])
                phiq_dbg = nc.dram_tensor("phiqdbg", [128, 2, SLAB], bf,
                                          kind="ExternalOutput")
                nc.sync.dma_start(out=phiq_dbg.ap(),
                                  in_=phiQ_sb[:, :, 0:SLAB])
                mbd_dbg = nc.dram_tensor("mbddbg", [128, 2, 128], f32,
                                         kind="ExternalOutput")
                mb_sb = singles.tile([128, 2, 128], f32)
                for t in range(2):
                    nc.vector.tensor_copy(mb_sb[:, t, :], M_bd[t][:])
                nc.sync.dma_start(out=mbd_dbg.ap(), in_=mb_sb[:])
    nc.compile()
    return nc


_NC_CACHE = None


def _get_nc(tau):
    global _NC_CACHE
    if _NC_CACHE is None:
        _NC_CACHE = _build_bass(tau)
    return _NC_CACHE


# ---------------------------------------------------------------------------
# host marshalling
# ---------------------------------------------------------------------------

def _consts():
    sqsel = np.zeros((128, 128), np.float32)
    sqsel[0:64, 0:64] = -0.5
    sqsel[64:128, 64:128] = -0.5
    id2w = 2.0 * np.eye(128, dtype=np.float32)
    return (sqsel.astype(BF16), id2w)


def _kernel_device(query, Wq, Wk, Wv, Wo, bo, tau, idx):
    global LAST_RESULTS
    from concourse.bass_utils import run_bass_kernel_spmd

    nc = _get_nc(tau)
    b, n, _ = query.shape

    sqsel, id2w = _consts()
    WkT = np.ascontiguousarray(Wk.T).astype(BF16)
    WvT = np.ascontiguousarray(Wv.T).astype(BF16)
    WoT = np.ascontiguousarray(Wo.T).astype(BF16)
    Wq_bf = Wq.astype(BF16)

    in_maps = []
    for bi in range(b):
        xT = np.ascontiguousarray(query[bi].T).astype(BF16)
        xlT = np.ascontiguousarray(query[bi][idx].T).astype(BF16)
        for g in range(GROUPS):
            sl = slice(g * C, (g + 1) * C)
            in_maps.append({
                "xT": xT,
                "xlT": xlT,
                "wkT": np.ascontiguousarray(WkT[:, sl]),
                "wq2": np.ascontiguousarray(Wq_bf[sl, :]),
                "wvT": np.ascontiguousarray(WvT[:, sl]),
                "woT": np.ascontiguousarray(WoT[sl, :]),
                "sqsel": sqsel,
                "id2w": id2w,
            })

    res = run_bass_kernel_spmd(nc, in_maps, core_ids=list(range(N_CORES)))
    LAST_RESULTS = res

    out = np.zeros((b, n, EMBED_DIM), np.float32)
    for bi in range(b):
        for g in range(GROUPS):
            out[bi] += res.results[bi * GROUPS + g]["opart"].astype(
                np.float32).T
    out += bo
    return out


def _kernel_numpy(query, Wq, bq, Wk, bk, Wv, bv, Wo, bo, tau, idx):
    """Reference-faithful fallback (nonzero biases etc.)."""
    b, n, _ = query.shape
    out = np.zeros((b, n, EMBED_DIM), np.float32)
    for bi in range(b):
        x = query[bi]
        Q = (x @ Wq.T + bq).reshape(n, NUM_HEADS, HEAD_DIM).transpose(1, 0, 2)
        K = (x @ Wk.T + bk).reshape(n, NUM_HEADS, HEAD_DIM).transpose(1, 0, 2)
        V = (x @ Wv.T + bv).reshape(n, NUM_HEADS, HEAD_DIM).transpose(1, 0, 2)
        L = K[:, idx, :]
        def sqd(X, Lm):
            Xn = np.sum(X * X, -1, keepdims=True)
            Ln = np.sum(Lm * Lm, -1, keepdims=True)
            return np.maximum(Xn + np.swapaxes(Ln, -2, -1)
                              - 2.0 * np.einsum("hnd,hkd->hnk", X, Lm), 0.0)
        PhiQ = np.exp(-sqd(Q, L) / tau)
        PhiK = np.exp(-sqd(K, L) / tau)
        Wk_ = np.exp(-sqd(L, L) / tau) + 1e-6 * np.eye(NL, dtype=np.float32)
        Z = np.einsum("hnk,hnd->hkd", PhiK, V)
        Y = np.linalg.solve(Wk_, Z)
        ou = np.einsum("hnk,hkd->hnd", PhiQ, Y)
        sY = np.linalg.solve(Wk_, PhiK.sum(1)[..., None])
        nrm = np.maximum(np.einsum("hnk,hko->hno", PhiQ, sY), 1e-10)
        H = (ou / nrm).transpose(1, 0, 2).reshape(n, EMBED_DIM)
        out[bi] = H @ Wo.T
    return out + bo


def kernel(query, Wq, bq, Wk, bk, Wv, bv, Wo, bo, temperature, landmark_idx):
    query = np.asarray(query, dtype=np.float32)
    Wq, Wk, Wv, Wo = (np.asarray(w, np.float32) for w in (Wq, Wk, Wv, Wo))
    bq, bk, bv, bo = (np.asarray(x, np.float32) for x in (bq, bk, bv, bo))
    tau = float(np.asarray(temperature))
    idx = np.asarray(landmark_idx).astype(np.int64)

    if (query.shape != (2, N, EMBED_DIM) or idx.shape != (NL,)
            or np.any(bq) or np.any(bk) or np.any(bv)):
        return _kernel_numpy(query, Wq, bq, Wk, bk, Wv, bv, Wo, bo, tau, idx)
    return _kernel_device(query, Wq, Wk, Wv, Wo, bo, tau, idx).astype(
        np.float32, copy=False)


# revision 12
# speedup vs baseline: 1.1462x; 1.0333x over previous
"""nn_ApproximateEuclideanAttention — 8-core Trainium2 Bass kernel (v2).

Sharding: data-parallel over batch (2) x tensor-parallel over heads (16 -> 4
groups of 4), one shard per NeuronCore. Each core computes its head-group's
attention and the partial output projection (written TRANSPOSED, embed-major);
the host sums the 4 partials per batch (+bo).

v2 restructure (validated numerically in val_v2.py, mean rel err 5.6e-3):
  - Q projection folded into the landmarks: S_Q = A_q x^T with
    A_q = blockdiag(L) Wq_pair, so phiQ comes from one accumulating matmul
    chain per pair (no QT eviction, no separate S_Q matmul).
  - Wo folded into Y: YT = Zp^T A (one matmul per pair, A symmetric), then
    Ypack = blockdiag(Y^T)^T wo_pair once; the per-slab H matmul disappears
    and the O-projection is outT_e += Ypack_e^T (phiQ * 1/norm).
  - norm broadcast: the norm matmul uses a blockdiag stationary selYE whose
    columns repeat sY per 64-row head block, directly producing the
    (128,seq) broadcast layout; ACT Reciprocal gives 1/norm in bf16. The
    old bsel broadcast matmuls + evictions disappear.
  - Z accumulates across all 16 slabs in one persistent PSUM bank (single
    start/stop pair; start=True clears has_written for the whole bank so
    only the very first matmul may carry it).
  - Startup: initial DMAs issued in parallel on sync+scalar+gpsimd queues,
    xlT early; junk warmup matmuls keep/get the PE HAM warm during the
    initial DMA wait.
  - Output DMA chunked (2 per slab) to shrink the end-of-kernel tail.
"""

import numpy as np
import ml_dtypes

EMBED_DIM = 1024
NUM_HEADS = 16
HEAD_DIM = 64
NL = 64          # landmarks
N_CORES = 8
GROUPS = 4       # head groups -> 4 heads / 256 cols per core
C = 256          # local channel cols per core
N = 8192         # sequence length per batch
SLAB = 512
NSLAB = N // SLAB          # 16
NCH = SLAB // 128          # chunks per slab = 4
NS_ITERS = 13
DEBUG_DUMP = False

BF16 = ml_dtypes.bfloat16

LAST_RESULTS = None  # BassKernelResults of the most recent device run


# ---------------------------------------------------------------------------
# device program
# ---------------------------------------------------------------------------

def _build_bass(tau: float):
    import concourse.bass as bass
    import concourse.tile as tile
    from concourse import bacc, mybir

    f32 = mybir.dt.float32
    f32r = mybir.dt.float32r
    bf = mybir.dt.bfloat16
    Exp = mybir.ActivationFunctionType.Exp
    Square = mybir.ActivationFunctionType.Square
    Copy = mybir.ActivationFunctionType.Copy

    nc = bacc.Bacc("TRN2", target_bir_lowering=False, debug=False,
                   num_devices=N_CORES)

    xT_d = nc.dram_tensor("xT", [EMBED_DIM, N], bf, kind="ExternalInput")
    xlT_d = nc.dram_tensor("xlT", [EMBED_DIM, NL], bf, kind="ExternalInput")
    wkT_d = nc.dram_tensor("wkT", [EMBED_DIM, C], bf, kind="ExternalInput")
    wq2_d = nc.dram_tensor("wq2", [C, EMBED_DIM], bf, kind="ExternalInput")
    wvT_d = nc.dram_tensor("wvT", [EMBED_DIM, C], bf, kind="ExternalInput")
    woT_d = nc.dram_tensor("woT", [C, EMBED_DIM], bf, kind="ExternalInput")
    sqsel_d = nc.dram_tensor("sqsel", [128, 128], bf, kind="ExternalInput")
    id2w_d = nc.dram_tensor("id2w", [128, 128], f32, kind="ExternalInput")
    out_d = nc.dram_tensor("opart", [EMBED_DIM, N], bf, kind="ExternalOutput")

    # dram views
    xT_r = xT_d.ap().rearrange("(ci p) n -> p ci n", p=128)       # (128,8,N)
    xlT_r = xlT_d.ap().rearrange("(ci p) l -> p ci l", p=128)     # (128,8,64)
    wk_r = wkT_d.ap().rearrange("(ci p) e -> p ci e", p=128)      # (128,8,256)
    wq2_r = wq2_d.ap().rearrange("(ct p) e -> p ct e", p=128)     # (128,2,1024)
    wv_r = wvT_d.ap().rearrange("(ci p) e -> p ci e", p=128)
    wo_r = woT_d.ap().rearrange("(ct p) e -> p ct e", p=128)      # (128,2,1024)
    out_rT = out_d.ap().rearrange("(e p) n -> p e n", p=128)      # (128,8,N)

    sc = 2.0 / tau
    XTS_BUFS = 4
    XTS_PREF = 2

    with tile.TileContext(nc) as tc:
        import contextlib
        ctx = contextlib.ExitStack()
        with ctx:
            singles = ctx.enter_context(tc.tile_pool(name="singles", bufs=1))
            big = ctx.enter_context(tc.tile_pool(name="big", bufs=1))
            slabs = ctx.enter_context(tc.tile_pool(name="slabs", bufs=2))
            nspool = ctx.enter_context(tc.tile_pool(name="nspool", bufs=2))
            ps_big = ctx.enter_context(
                tc.tile_pool(name="ps_big", bufs=2, space="PSUM"))
            ps_v = ctx.enter_context(
                tc.tile_pool(name="ps_v", bufs=2, space="PSUM"))
            ps_e = ctx.enter_context(
                tc.tile_pool(name="ps_e", bufs=2, space="PSUM"))
            ps_ns = ctx.enter_context(
                tc.tile_pool(name="ps_ns", bufs=1, space="PSUM"))
            ps_z = ctx.enter_context(
                tc.tile_pool(name="ps_z", bufs=1, space="PSUM"))

            # ---- stage 0: weights + consts, spread across DMA queues -----
            wk_sb = singles.tile([128, 8, C], bf)
            wq2_sb = singles.tile([128, 2, EMBED_DIM], bf)
            wv_sb = singles.tile([128, 8, C], bf)
            wo_sb = singles.tile([128, 2, EMBED_DIM], bf)
            sqsel_sb = singles.tile([128, 128], bf)
            id2w_sb = singles.tile([128, 128], f32)
            xlT_sb = singles.tile([128, 8, NL], bf)

            # PE warmup: junk matmuls with no DMA deps; run during the
            # initial DMA wait so HAM reaches K=8/8 before real work.
            junk = singles.tile([128, 256], bf)
            nc.vector.memset(junk[:], 0.0)
            warm_ps = ps_ns.tile([128, 256], f32, tag="nsp", name="warmps")
            for i in range(16):
                nc.tensor.matmul(warm_ps[:], junk[:, 0:128], junk[:],
                                 start=(i == 0), stop=(i == 15))

            nc.sync.dma_start(out=wk_sb[:], in_=wk_r)
            nc.scalar.dma_start(out=xlT_sb[:], in_=xlT_r)
            xts_pref = {}
            for s in range(XTS_PREF):
                xpf = slabs.tile([128, 8, SLAB], bf, tag="xts", bufs=XTS_BUFS,
                                 name=f"xtspref{s}")
                nc.sync.dma_start(out=xpf[:],
                                  in_=xT_r[:, :, s * SLAB:(s + 1) * SLAB])
                xts_pref[s] = xpf
            nc.scalar.dma_start(out=wq2_sb[:], in_=wq2_r)
            nc.scalar.dma_start(out=wv_sb[:], in_=wv_r)
            nc.scalar.dma_start(out=wo_sb[:], in_=wo_r)
            nc.gpsimd.dma_start(out=sqsel_sb[:], in_=sqsel_d.ap())
            nc.gpsimd.dma_start(out=id2w_sb[:], in_=id2w_d.ap())

            # absorb the const-bias-AP DMA wait into one tiny ACT op
            warm1 = singles.tile([1, 1], f32)
            nc.scalar.activation(warm1[:], id2w_sb[0:1, 0:1], Exp)

            # ---- stage 1: landmarks L^T (256, 64), chan on partitions ----
            LT_sb = singles.tile([128, 2, NL], bf)
            for co in range(2):
                L_ps = ps_big.tile([128, NL], f32, tag="psb")
                for ci in range(8):
                    nc.tensor.matmul(
                        L_ps[:], wk_sb[:, ci, co * 128:(co + 1) * 128],
                        xlT_sb[:, ci, :], start=(ci == 0), stop=(ci == 7))
                nc.vector.tensor_copy(LT_sb[:, co, :], L_ps[:])

            # blockdiag(L^T) per pair (chan-part, land-cols)
            bdl = singles.tile([128, 2, 128], bf)
            nc.vector.memset(bdl[:], 0.0)
            for t in range(2):
                nc.vector.tensor_copy(bdl[0:64, t, 0:64], LT_sb[0:64, t, :])
                nc.vector.tensor_copy(bdl[64:128, t, 64:128], LT_sb[64:128, t, :])

            # ---- stage 1b: A_qT = wq2_pair^T-contract @ bdl  (Q folding) --
            A_qT = singles.tile([128, 8, 2, 128], bf)
            for t in range(2):
                for ch in range(0, 8, 2):
                    Aq_ps = ps_big.tile([128, 256], f32, tag="psb",
                                        name=f"aqps{t}_{ch}")
                    for k in range(2):
                        ci = ch + k
                        nc.tensor.matmul(
                            Aq_ps[:, 128 * k:128 * k + 128],
                            wq2_sb[:, t, ci * 128:(ci + 1) * 128],
                            bdl[:, t, :], start=(k == 0), stop=(k == 1),
                            skip_group_check=True)
                    for k in range(2):
                        nc.vector.tensor_copy(
                            A_qT[:, ch + k, t, :],
                            Aq_ps[:, 128 * k:128 * k + 128])

            # ---- stage 2: Wexp + Newton-Schulz inverse (f32r matmuls) ----
            W_ps = ps_big.tile([128, 128], f32, tag="psb")
            for t in range(2):
                nc.tensor.matmul(W_ps[:, 64 * t:64 * t + 64],
                                 bdl[:, t, :], LT_sb[:, t, :])
            Wf_sb = singles.tile([128, 128], f32)
            nc.scalar.activation(Wf_sb[:], W_ps[:], Exp, scale=sc)
            W_bd = [singles.tile([128, 128], f32, tag=f"wbd{t}", name=f"wbd{t}")
                    for t in range(2)]
            for t in range(2):
                nc.vector.memset(W_bd[t][:], 0.0)
                nc.vector.tensor_copy(W_bd[t][0:64, 0:64],
                                      Wf_sb[0:64, 64 * t:64 * t + 64])
                nc.vector.tensor_copy(W_bd[t][64:128, 64:128],
                                      Wf_sb[64:128, 64 * t:64 * t + 64])

            X_ping = [singles.tile([128, 128], f32, tag=f"xa{t}", name=f"xa{t}")
                      for t in range(2)]
            X_pong = [singles.tile([128, 128], f32, tag=f"xb{t}", name=f"xb{t}")
                      for t in range(2)]
            for t in range(2):
                nc.vector.tensor_scalar_mul(X_ping[t][:], id2w_sb[:],
                                            1.0 / 256.0)
            cur, nxt = X_ping, X_pong
            for it in range(NS_ITERS):
                for t in range(2):
                    P_ps = ps_ns.tile([128, 128], f32, tag="nsp")
                    nc.tensor.matmul(P_ps[:], W_bd[t][:], cur[t][:])
                    G_sb = nspool.tile([128, 128], f32, tag="nsg")
                    nc.vector.tensor_sub(G_sb[:], id2w_sb[:], P_ps[:])
                    Xp_ps = ps_ns.tile([128, 128], f32, tag="nsp")
                    nc.tensor.matmul(Xp_ps[:], cur[t][:], G_sb[:])
                    nc.vector.tensor_copy(nxt[t][:], Xp_ps[:])
                cur, nxt = nxt, cur
            M_bd = cur  # fp32 blockdiag inverse per pair

            # ---- stage 3: streaming projections + Phi + Z ----------------
            phiQ_sb = big.tile([128, 2, N], bf)
            Z_ps = ps_z.tile([128, 258], f32, tag="zacc")  # persistent bank

            def emit_sq(s, xts):
                nsl = slice(s * SLAB, (s + 1) * SLAB)
                for t in range(2):
                    SQ_ps = ps_big.tile([128, SLAB], f32, tag="psb",
                                        name=f"sqps{s}_{t}")
                    for ci in range(8):
                        nc.tensor.matmul(SQ_ps[:], A_qT[:, ci, t, :],
                                         xts[:, ci, :], start=(ci == 0),
                                         stop=(ci == 7))
                    nc.scalar.activation(phiQ_sb[:, t, nsl], SQ_ps[:], Exp,
                                         scale=sc)

            xts_keep = {}
            for s in range(NSLAB):
                sp = s + XTS_PREF
                if sp < NSLAB:
                    xpf = slabs.tile([128, 8, SLAB], bf, tag="xts",
                                     bufs=XTS_BUFS, name=f"xts{sp}")
                    nc.sync.dma_start(out=xpf[:],
                                      in_=xT_r[:, :, sp * SLAB:(sp + 1) * SLAB])
                    xts_pref[sp] = xpf
                xts = xts_pref.pop(s)

                # K^T and squares
                KT = slabs.tile([128, 2, SLAB], bf, tag="kt", bufs=3)
                sqKT = slabs.tile([128, 2, SLAB], bf, tag="sqkt", bufs=3)
                for co in range(2):
                    K_ps = ps_big.tile([128, SLAB], f32, tag="psb")
                    for ci in range(8):
                        nc.tensor.matmul(
                            K_ps[:], wk_sb[:, ci, co * 128:(co + 1) * 128],
                            xts[:, ci, :], start=(ci == 0), stop=(ci == 7))
                    nc.vector.tensor_copy(KT[:, co, :], K_ps[:])
                    nc.scalar.activation(sqKT[:, co, :], K_ps[:], Square)

                # phiQ (folded Q projection); slabs 14/15 deferred to
                # stage 4 to fill the PE during the serial solve section
                if s < NSLAB - 2:
                    emit_sq(s, xts)
                else:
                    xts_keep[s] = xts

                # V (N-land), half-slab psums; Vb laid out per (chunk,
                # pair) as [V(128) | ones(1)].
                Vb = slabs.tile([128, NCH, 2 * 129], bf, tag="vb", bufs=3)
                nc.vector.memset(Vb[:, :, 128:129], 1.0)
                nc.vector.memset(Vb[:, :, 257:258], 1.0)
                for hf in range(2):
                    V_ps = ps_v.tile([128, 2 * C], f32, tag="psv")
                    for c in (2 * hf, 2 * hf + 1):
                        for ci in range(8):
                            nc.tensor.matmul(
                                V_ps[:, (c % 2) * C:(c % 2 + 1) * C],
                                xts[:, ci, c * 128:(c + 1) * 128],
                                wv_sb[:, ci, :], start=(ci == 0),
                                stop=(ci == 7))
                    for c in (2 * hf, 2 * hf + 1):
                        for t in range(2):
                            nc.vector.tensor_copy(
                                Vb[:, c, 129 * t:129 * t + 128],
                                V_ps[:, (c % 2) * C + 128 * t:
                                     (c % 2) * C + 128 * t + 128])

                # E_K (N-land) -> Phi_K', half-slab psums
                phiK = slabs.tile([128, NCH, C], bf, tag="phik", bufs=3)
                for hf in range(2):
                    E_ps = ps_e.tile([128, 2 * C], f32, tag="pse")
                    for c in (2 * hf, 2 * hf + 1):
                        for t in range(2):
                            cs = slice((c % 2) * C + 128 * t,
                                       (c % 2) * C + 128 * t + 128)
                            nc.tensor.matmul(E_ps[:, cs],
                                             KT[:, t, c * 128:(c + 1) * 128],
                                             bdl[:, t, :], start=True,
                                             stop=False)
                            nc.tensor.matmul(E_ps[:, cs],
                                             sqKT[:, t, c * 128:(c + 1) * 128],
                                             sqsel_sb[:], start=False,
                                             stop=True)
                    nc.scalar.activation(phiK[:, 2 * hf:2 * hf + 2, :],
                                         E_ps[:], Exp, scale=sc)

                # Z accumulated across ALL slabs in the persistent PSUM
                # bank: exactly one start (s==0 first mm) / stop (last).
                for c in range(NCH):
                    for t in range(2):
                        zc = 129 * t
                        nc.tensor.matmul(
                            Z_ps[:, zc:zc + 129],
                            phiK[:, c, 128 * t:128 * t + 128],
                            Vb[:, c, 129 * t:129 * t + 129],
                            start=(s == 0 and c == 0 and t == 0),
                            stop=(s == NSLAB - 1 and c == NCH - 1 and t == 1),
                            skip_group_check=True)

            # ---- stage 4: solve application (transposed), Wo folding -----
            # Zp pack from the persistent Z bank: cols [65t:65t+64] rows
            # 0:64 <- Z0 diag-block, rows 64:128 <- Z1 diag-block; col
            # 65t+64 <- s' (both halves valid).
            Zp_sb = singles.tile([128, 130], f32)
            for t in range(2):
                nc.vector.tensor_copy(Zp_sb[0:64, 65 * t:65 * t + 64],
                                      Z_ps[0:64, 129 * t:129 * t + 64])
                nc.vector.tensor_copy(Zp_sb[64:128, 65 * t:65 * t + 64],
                                      Z_ps[64:128, 129 * t + 64:129 * t + 128])
                nc.vector.tensor_copy(Zp_sb[:, 65 * t + 64:65 * t + 65],
                                      Z_ps[:, 129 * t + 128:129 * t + 129])

            # YT_pair = Zp_slice^T @ M_bd  (65 part: 64 chan + sY row? no —
            # rows = 64 chan cols of Zp slice + the s' col -> row 64 unused
            # here; Y^T blocks live in rows 0:64, cols [land0|land1]).
            YT_ps = []
            for t in range(2):
                yt = ps_big.tile([128, 128], f32, tag="psb", name=f"ytps{t}")
                nc.tensor.matmul(yt[0:65, :],
                                 Zp_sb[:, 65 * t:65 * t + 65],
                                 M_bd[t][:])
                YT_ps.append(yt)

            # sY = A @ s' (orig orientation, partition-dim vector per pair)
            sY_ps = ps_ns.tile([128, 2], f32, tag="nsp", name="syps")
            for t in range(2):
                nc.tensor.matmul(
                    sY_ps[:, t:t + 1], M_bd[t][:],
                    Zp_sb[:, 65 * t + 64:65 * t + 65],
                    skip_group_check=True)

            # deferred slab-14 phiQ fills the PE while the DVE does the
            # stage-4 packing / selYE construction
            emit_sq(NSLAB - 2, xts_keep[NSLAB - 2])

            # Y_bdT (chan-part, land-cols) blockdiag, bf16
            Y_bdT = singles.tile([128, 2, 128], bf)
            nc.vector.memset(Y_bdT[:], 0.0)
            for t in range(2):
                nc.vector.tensor_copy(Y_bdT[0:64, t, 0:64],
                                      YT_ps[t][0:64, 0:64])
                nc.vector.tensor_copy(Y_bdT[64:128, t, 64:128],
                                      YT_ps[t][0:64, 64:128])

            # Ypack[t] = Y_bdT[t]^T-contract @ wo_pair  (land-part, embed)
            Ypack = singles.tile([128, 2, EMBED_DIM], bf)
            for t in range(2):
                for eh in range(2):
                    yp_ps = ps_ns.tile([128, 512], f32, tag="nsp",
                                       name=f"ypps{t}_{eh}")
                    nc.tensor.matmul(yp_ps[:], Y_bdT[:, t, :],
                                     wo_sb[:, t, eh * 512:eh * 512 + 512])
                    nc.vector.tensor_copy(
                        Ypack[:, t, eh * 512:eh * 512 + 512], yp_ps[:])

            emit_sq(NSLAB - 1, xts_keep[NSLAB - 1])

            # selYE[t]: blockdiag broadcast of sY per 64-row head block
            selYE = singles.tile([128, 2, 128], bf)
            nc.vector.memset(selYE[:], 0.0)
            for t in range(2):
                for hh in range(2):
                    r = slice(64 * hh, 64 * hh + 64)
                    nc.vector.tensor_copy(
                        selYE[r, t, 64 * hh:64 * hh + 64],
                        sY_ps[r, t:t + 1].broadcast_to([64, 64]))

            # ---- stage 5: norm -> 1/norm -> phiQn -> transposed O-proj ---
            for s in range(NSLAB):
                nsl = slice(s * SLAB, (s + 1) * SLAB)
                rnE = slabs.tile([128, 2, SLAB], f32, tag="rne", bufs=3)
                phiQn = slabs.tile([128, 2, SLAB], bf, tag="pqn", bufs=3)
                for t in range(2):
                    n_ps = ps_big.tile([128, SLAB], f32, tag="psb",
                                       name=f"nps{s}_{t}")
                    nc.tensor.matmul(n_ps[:], selYE[:, t, :],
                                     phiQ_sb[:, t, nsl])
                    nc.vector.reciprocal_approx_fast(out=rnE[:, t, :],
                                                     in_=n_ps[:])
                    nc.vector.tensor_mul(phiQn[:, t, :], phiQ_sb[:, t, nsl],
                                         rnE[:, t, :])

                oout = slabs.tile([128, 8, SLAB], bf, tag="oout", bufs=3)
                for e in range(8):
                    pool = ps_v if e % 2 == 0 else ps_e
                    O_ps = pool.tile([128, SLAB], f32,
                                     tag=("psv" if e % 2 == 0 else "pse"),
                                     name=f"ops{s}_{e}")
                    for t in range(2):
                        nc.tensor.matmul(
                            O_ps[:], Ypack[:, t, e * 128:(e + 1) * 128],
                            phiQn[:, t, :], start=(t == 0), stop=(t == 1))
                    if e < 2:
                        nc.vector.tensor_copy(oout[:, e, :], O_ps[:])
                    else:
                        nc.scalar.activation(oout[:, e, :], O_ps[:], Copy)
                for q in range(4):
                    nc.sync.dma_start(
                        out=out_rT[:, 2 * q:2 * q + 2, nsl],
                        in_=oout[:, 2 * q:2 * q + 2, :])
                if DEBUG_DUMP and s == 0:
                    rne_dbg = nc.dram_tensor("rnedbg", [128, 2, SLAB], f32,
                                             kind="ExternalOutput")
                    pqn_dbg = nc.dram_tensor("pqndbg", [128, 2, SLAB], bf,
                                             kind="ExternalOutput")
                    nc.sync.dma_start(out=rne_dbg.ap(), in_=rnE[:])
                    nc.sync.dma_start(out=pqn_dbg.ap(), in_=phiQn[:])

            if DEBUG_DUMP:
                z_dbg = nc.dram_tensor("zdbg", [128, 258], f32,
                                       kind="ExternalOutput")
                zc_sb = singles.tile([128, 258], f32)
                nc.vector.tensor_copy(zc_sb[:], Z_ps[:])
                nc.sync.dma_start(out=z_dbg.ap(), in_=zc_sb[:])
                zp_dbg = nc.dram_tensor("zpdbg", [128, 130], f32,
                                        kind="ExternalOutput")
                nc.sync.dma_start(out=zp_dbg.ap(), in_=Zp_sb[:])
                yt_dbg = nc.dram_tensor("ytdbg", [128, 2, 128], f32,
                                        kind="ExternalOutput")
                yt_sb = singles.tile([128, 2, 128], f32)
                for t in range(2):
                    nc.vector.tensor_copy(yt_sb[:, t, :], YT_ps[t][:])
                nc.sync.dma_start(out=yt_dbg.ap(), in_=yt_sb[:])
                sy_dbg = nc.dram_tensor("sydbg", [128, 2], f32,
                                        kind="ExternalOutput")
                sy_sb = singles.tile([128, 2], f32)
                nc.vector.tensor_copy(sy_sb[:], sY_ps[:])
                nc.sync.dma_start(out=sy_dbg.ap(), in_=sy_sb[:])
                sel_dbg = nc.dram_tensor("seldbg", [128, 2, 128], bf,
                                         kind="ExternalOutput")
                nc.sync.dma_start(out=sel_dbg.ap(), in_=selYE[:])
                yp_dbg = nc.dram_tensor("ypdbg", [128, 2, 1024], bf,
                                        kind="ExternalOutput")
                nc.sync.dma_start(out=yp_dbg.ap(), in_=Ypack[:])
                ybdt_dbg = nc.dram_tensor("ybdtdbg", [128, 2, 128], bf,
                                          kind="ExternalOutput")
                nc.sync.dma_start(out=ybdt_dbg.ap(), in_=Y_bdT[:])
                phiq_dbg = nc.dram_tensor("phiqdbg", [128, 2, SLAB], bf,
                                          kind="ExternalOutput")
                nc.sync.dma_start(out=phiq_dbg.ap(),
                                  in_=phiQ_sb[:, :, 0:SLAB])
                mbd_dbg = nc.dram_tensor("mbddbg", [128, 2, 128], f32,
                                         kind="ExternalOutput")
                mb_sb = singles.tile([128, 2, 128], f32)
                for t in range(2):
                    nc.vector.tensor_copy(mb_sb[:, t, :], M_bd[t][:])
                nc.sync.dma_start(out=mbd_dbg.ap(), in_=mb_sb[:])
                xw_dbg = nc.dram_tensor("xwdbg", [128, 2, 128], f32,
                                        kind="ExternalOutput")
                xw_sb = singles.tile([128, 2, 128], f32)
                for t in range(2):
                    nc.vector.tensor_copy(xw_sb[:, t, :], cur[t][:])
                nc.sync.dma_start(out=xw_dbg.ap(), in_=xw_sb[:])
                wbd_dbg = nc.dram_tensor("wbddbg", [128, 2, 128], f32,
                                         kind="ExternalOutput")
                wb_sb = singles.tile([128, 2, 128], f32)
                for t in range(2):
                    nc.vector.tensor_copy(wb_sb[:, t, :], W_bd[t][:])
                nc.sync.dma_start(out=wbd_dbg.ap(), in_=wb_sb[:])
    nc.compile()
    return nc


_NC_CACHE = None


def _get_nc(tau):
    global _NC_CACHE
    if _NC_CACHE is None:
        _NC_CACHE = _build_bass(tau)
    return _NC_CACHE


# ---------------------------------------------------------------------------
# host marshalling
# ---------------------------------------------------------------------------

def _consts():
    sqsel = np.zeros((128, 128), np.float32)
    sqsel[0:64, 0:64] = -0.5
    sqsel[64:128, 64:128] = -0.5
    id2w = 2.0 * np.eye(128, dtype=np.float32)
    return (sqsel.astype(BF16), id2w)


def _kernel_device(query, Wq, Wk, Wv, Wo, bo, tau, idx):
    global LAST_RESULTS
    from concourse.bass_utils import run_bass_kernel_spmd

    nc = _get_nc(tau)
    b, n, _ = query.shape

    sqsel, id2w = _consts()
    WkT = np.ascontiguousarray(Wk.T).astype(BF16)
    WvT = np.ascontiguousarray(Wv.T).astype(BF16)
    WoT = np.ascontiguousarray(Wo.T).astype(BF16)
    Wq_bf = Wq.astype(BF16)

    in_maps = []
    for bi in range(b):
        xT = np.ascontiguousarray(query[bi].T).astype(BF16)
        xlT = np.ascontiguousarray(query[bi][idx].T).astype(BF16)
        for g in range(GROUPS):
            sl = slice(g * C, (g + 1) * C)
            in_maps.append({
                "xT": xT,
                "xlT": xlT,
                "wkT": np.ascontiguousarray(WkT[:, sl]),
                "wq2": np.ascontiguousarray(Wq_bf[sl, :]),
                "wvT": np.ascontiguousarray(WvT[:, sl]),
                "woT": np.ascontiguousarray(WoT[sl, :]),
                "sqsel": sqsel,
                "id2w": id2w,
            })

    res = run_bass_kernel_spmd(nc, in_maps, core_ids=list(range(N_CORES)))
    LAST_RESULTS = res

    out = np.zeros((b, n, EMBED_DIM), np.float32)
    for bi in range(b):
        for g in range(GROUPS):
            out[bi] += res.results[bi * GROUPS + g]["opart"].astype(
                np.float32).T
    out += bo
    return out


def _kernel_numpy(query, Wq, bq, Wk, bk, Wv, bv, Wo, bo, tau, idx):
    """Reference-faithful fallback (nonzero biases etc.)."""
    b, n, _ = query.shape
    out = np.zeros((b, n, EMBED_DIM), np.float32)
    for bi in range(b):
        x = query[bi]
        Q = (x @ Wq.T + bq).reshape(n, NUM_HEADS, HEAD_DIM).transpose(1, 0, 2)
        K = (x @ Wk.T + bk).reshape(n, NUM_HEADS, HEAD_DIM).transpose(1, 0, 2)
        V = (x @ Wv.T + bv).reshape(n, NUM_HEADS, HEAD_DIM).transpose(1, 0, 2)
        L = K[:, idx, :]
        def sqd(X, Lm):
            Xn = np.sum(X * X, -1, keepdims=True)
            Ln = np.sum(Lm * Lm, -1, keepdims=True)
            return np.maximum(Xn + np.swapaxes(Ln, -2, -1)
                              - 2.0 * np.einsum("hnd,hkd->hnk", X, Lm), 0.0)
        PhiQ = np.exp(-sqd(Q, L) / tau)
        PhiK = np.exp(-sqd(K, L) / tau)
        Wk_ = np.exp(-sqd(L, L) / tau) + 1e-6 * np.eye(NL, dtype=np.float32)
        Z = np.einsum("hnk,hnd->hkd", PhiK, V)
        Y = np.linalg.solve(Wk_, Z)
        ou = np.einsum("hnk,hkd->hnd", PhiQ, Y)
        sY = np.linalg.solve(Wk_, PhiK.sum(1)[..., None])
        nrm = np.maximum(np.einsum("hnk,hko->hno", PhiQ, sY), 1e-10)
        H = (ou / nrm).transpose(1, 0, 2).reshape(n, EMBED_DIM)
        out[bi] = H @ Wo.T
    return out + bo


def kernel(query, Wq, bq, Wk, bk, Wv, bv, Wo, bo, temperature, landmark_idx):
    query = np.asarray(query, dtype=np.float32)
    Wq, Wk, Wv, Wo = (np.asarray(w, np.float32) for w in (Wq, Wk, Wv, Wo))
    bq, bk, bv, bo = (np.asarray(x, np.float32) for x in (bq, bk, bv, bo))
    tau = float(np.asarray(temperature))
    idx = np.asarray(landmark_idx).astype(np.int64)

    if (query.shape != (2, N, EMBED_DIM) or idx.shape != (NL,)
            or np.any(bq) or np.any(bk) or np.any(bv)):
        return _kernel_numpy(query, Wq, bq, Wk, bk, Wv, bv, Wo, bo, tau, idx)
    return _kernel_device(query, Wq, Wk, Wv, Wo, bo, tau, idx).astype(
        np.float32, copy=False)


# revision 14
# speedup vs baseline: 1.3532x; 1.1806x over previous
"""nn_ApproximateEuclideanAttention — 8-core Trainium2 Bass kernel (v2).

Sharding: data-parallel over batch (2) x tensor-parallel over heads (16 -> 4
groups of 4), one shard per NeuronCore. Each core computes its head-group's
attention and the partial output projection (written TRANSPOSED, embed-major);
the host sums the 4 partials per batch (+bo).

v2 restructure (validated numerically in val_v2.py, mean rel err 5.6e-3):
  - Q projection folded into the landmarks: S_Q = A_q x^T with
    A_q = blockdiag(L) Wq_pair, so phiQ comes from one accumulating matmul
    chain per pair (no QT eviction, no separate S_Q matmul).
  - Wo folded into Y: YT = Zp^T A (one matmul per pair, A symmetric), then
    Ypack = blockdiag(Y^T)^T wo_pair once; the per-slab H matmul disappears
    and the O-projection is outT_e += Ypack_e^T (phiQ * 1/norm).
  - norm broadcast: the norm matmul uses a blockdiag stationary selYE whose
    columns repeat sY per 64-row head block, directly producing the
    (128,seq) broadcast layout; ACT Reciprocal gives 1/norm in bf16. The
    old bsel broadcast matmuls + evictions disappear.
  - Z accumulates across all 16 slabs in one persistent PSUM bank (single
    start/stop pair; start=True clears has_written for the whole bank so
    only the very first matmul may carry it).
  - Startup: initial DMAs issued in parallel on sync+scalar+gpsimd queues,
    xlT early; junk warmup matmuls keep/get the PE HAM warm during the
    initial DMA wait.
  - Output DMA chunked (2 per slab) to shrink the end-of-kernel tail.
"""

import numpy as np
import ml_dtypes

EMBED_DIM = 1024
NUM_HEADS = 16
HEAD_DIM = 64
NL = 64          # landmarks
N_CORES = 8
GROUPS = 4       # head groups -> 4 heads / 256 cols per core
C = 256          # local channel cols per core
N = 8192         # sequence length per batch
SLAB = 512
NSLAB = N // SLAB          # 16
NCH = SLAB // 128          # chunks per slab = 4
NS_ITERS = 13
DEBUG_DUMP = False

BF16 = ml_dtypes.bfloat16

LAST_RESULTS = None  # BassKernelResults of the most recent device run


# ---------------------------------------------------------------------------
# device program
# ---------------------------------------------------------------------------

def _build_bass(tau: float):
    import concourse.bass as bass
    import concourse.tile as tile
    from concourse import bacc, mybir

    f32 = mybir.dt.float32
    f8e4 = mybir.dt.float8e4
    DR = mybir.MatmulPerfMode.DoubleRow
    bf = mybir.dt.bfloat16
    Exp = mybir.ActivationFunctionType.Exp
    Square = mybir.ActivationFunctionType.Square
    Copy = mybir.ActivationFunctionType.Copy

    nc = bacc.Bacc("TRN2", target_bir_lowering=False, debug=False,
                   num_devices=N_CORES)

    xT_d = nc.dram_tensor("xT", [EMBED_DIM, N], bf, kind="ExternalInput")
    xlT_d = nc.dram_tensor("xlT", [EMBED_DIM, NL], bf, kind="ExternalInput")
    wkT_d = nc.dram_tensor("wkT", [EMBED_DIM, C], bf, kind="ExternalInput")
    x8_d = nc.dram_tensor("x8", [EMBED_DIM, N], f8e4, kind="ExternalInput")
    wk8_d = nc.dram_tensor("wk8", [EMBED_DIM, C], f8e4, kind="ExternalInput")
    aq8_d = nc.dram_tensor("aq8", [128, 8, 2, 128], f8e4,
                           kind="ExternalInput")
    wvT_d = nc.dram_tensor("wvT", [EMBED_DIM, C], bf, kind="ExternalInput")
    woT_d = nc.dram_tensor("woT", [C, EMBED_DIM], bf, kind="ExternalInput")
    sqsel_d = nc.dram_tensor("sqsel", [128, 128], bf, kind="ExternalInput")
    id2w_d = nc.dram_tensor("id2w", [128, 128], f32, kind="ExternalInput")
    out_d = nc.dram_tensor("opart", [EMBED_DIM, N], bf, kind="ExternalOutput")

    # dram views
    xT_r = xT_d.ap().rearrange("(ci p) n -> p ci n", p=128)       # (128,8,N)
    xlT_r = xlT_d.ap().rearrange("(ci p) l -> p ci l", p=128)     # (128,8,64)
    wk_r = wkT_d.ap().rearrange("(ci p) e -> p ci e", p=128)      # (128,8,256)
    x8_r = x8_d.ap().rearrange("(ci p) n -> p ci n", p=128)       # (128,8,N)
    wk8_r = wk8_d.ap().rearrange("(ci p) e -> p ci e", p=128)     # (128,8,256)
    wv_r = wvT_d.ap().rearrange("(ci p) e -> p ci e", p=128)
    wo_r = woT_d.ap().rearrange("(ct p) e -> p ct e", p=128)      # (128,2,1024)
    out_rT = out_d.ap().rearrange("(e p) n -> p e n", p=128)      # (128,8,N)

    sc = 2.0 / tau
    XTS_BUFS = 4
    XTS_PREF = 2

    with tile.TileContext(nc) as tc:
        import contextlib
        ctx = contextlib.ExitStack()
        with ctx:
            singles = ctx.enter_context(tc.tile_pool(name="singles", bufs=1))
            big = ctx.enter_context(tc.tile_pool(name="big", bufs=1))
            slabs = ctx.enter_context(tc.tile_pool(name="slabs", bufs=2))
            nspool = ctx.enter_context(tc.tile_pool(name="nspool", bufs=2))
            ps_big = ctx.enter_context(
                tc.tile_pool(name="ps_big", bufs=2, space="PSUM"))
            ps_v = ctx.enter_context(
                tc.tile_pool(name="ps_v", bufs=2, space="PSUM"))
            ps_e = ctx.enter_context(
                tc.tile_pool(name="ps_e", bufs=2, space="PSUM"))
            ps_ns = ctx.enter_context(
                tc.tile_pool(name="ps_ns", bufs=1, space="PSUM"))
            ps_z = ctx.enter_context(
                tc.tile_pool(name="ps_z", bufs=1, space="PSUM"))

            # ---- stage 0: weights + consts, spread across DMA queues -----
            wk_sb = singles.tile([128, 8, C], bf)
            wk8_sb = singles.tile([128, 8, C], f8e4)
            aq8_sb = singles.tile([128, 8, 2, 128], f8e4)
            wv_sb = singles.tile([128, 8, C], bf)
            wo_sb = singles.tile([128, 2, EMBED_DIM], bf)
            sqsel_sb = singles.tile([128, 128], bf)
            id2w_sb = singles.tile([128, 128], f32)
            xlT_sb = singles.tile([128, 8, NL], bf)

            # PE warmup: junk matmuls with no DMA deps; run during the
            # initial DMA wait so HAM reaches K=8/8 before real work.
            junk = singles.tile([128, 256], bf)
            nc.vector.memset(junk[:], 0.0)
            warm_ps = ps_ns.tile([128, 256], f32, tag="nsp", name="warmps")
            for i in range(16):
                nc.tensor.matmul(warm_ps[:], junk[:, 0:128], junk[:],
                                 start=(i == 0), stop=(i == 15))

            nc.sync.dma_start(out=wk_sb[:], in_=wk_r)
            nc.scalar.dma_start(out=xlT_sb[:], in_=xlT_r)
            xts_pref = {}
            x8_pref = {}
            for s in range(XTS_PREF):
                xpf = slabs.tile([128, 8, SLAB], bf, tag="xts", bufs=XTS_BUFS,
                                 name=f"xtspref{s}")
                nc.sync.dma_start(out=xpf[:],
                                  in_=xT_r[:, :, s * SLAB:(s + 1) * SLAB])
                xts_pref[s] = xpf
                x8pf = slabs.tile([128, 8, SLAB], f8e4, tag="x8ts",
                                  bufs=XTS_BUFS, name=f"x8tspref{s}")
                nc.scalar.dma_start(out=x8pf[:],
                                    in_=x8_r[:, :, s * SLAB:(s + 1) * SLAB])
                x8_pref[s] = x8pf
            nc.scalar.dma_start(out=wk8_sb[:], in_=wk8_r)
            nc.scalar.dma_start(out=aq8_sb[:], in_=aq8_d.ap())
            nc.scalar.dma_start(out=wv_sb[:], in_=wv_r)
            nc.scalar.dma_start(out=wo_sb[:], in_=wo_r)
            nc.gpsimd.dma_start(out=sqsel_sb[:], in_=sqsel_d.ap())
            nc.gpsimd.dma_start(out=id2w_sb[:], in_=id2w_d.ap())

            # absorb the const-bias-AP DMA wait into one tiny ACT op
            warm1 = singles.tile([1, 1], f32)
            nc.scalar.activation(warm1[:], id2w_sb[0:1, 0:1], Exp)

            # ---- stage 1: landmarks L^T (256, 64), chan on partitions ----
            LT_sb = singles.tile([128, 2, NL], bf)
            for co in range(2):
                L_ps = ps_big.tile([128, NL], f32, tag="psb")
                for ci in range(8):
                    nc.tensor.matmul(
                        L_ps[:], wk_sb[:, ci, co * 128:(co + 1) * 128],
                        xlT_sb[:, ci, :], start=(ci == 0), stop=(ci == 7))
                nc.vector.tensor_copy(LT_sb[:, co, :], L_ps[:])

            # blockdiag(L^T) per pair (chan-part, land-cols)
            bdl = singles.tile([128, 2, 128], bf)
            nc.vector.memset(bdl[:], 0.0)
            for t in range(2):
                nc.vector.tensor_copy(bdl[0:64, t, 0:64], LT_sb[0:64, t, :])
                nc.vector.tensor_copy(bdl[64:128, t, 64:128], LT_sb[64:128, t, :])

            # ---- stage 2: Wexp + Newton-Schulz inverse (f32r matmuls) ----
            W_ps = ps_big.tile([128, 128], f32, tag="psb")
            for t in range(2):
                nc.tensor.matmul(W_ps[:, 64 * t:64 * t + 64],
                                 bdl[:, t, :], LT_sb[:, t, :])
            Wf_sb = singles.tile([128, 128], f32)
            nc.scalar.activation(Wf_sb[:], W_ps[:], Exp, scale=sc)
            W_bd = [singles.tile([128, 128], f32, tag=f"wbd{t}", name=f"wbd{t}")
                    for t in range(2)]
            for t in range(2):
                nc.vector.memset(W_bd[t][:], 0.0)
                nc.vector.tensor_copy(W_bd[t][0:64, 0:64],
                                      Wf_sb[0:64, 64 * t:64 * t + 64])
                nc.vector.tensor_copy(W_bd[t][64:128, 64:128],
                                      Wf_sb[64:128, 64 * t:64 * t + 64])

            X_ping = [singles.tile([128, 128], f32, tag=f"xa{t}", name=f"xa{t}")
                      for t in range(2)]
            X_pong = [singles.tile([128, 128], f32, tag=f"xb{t}", name=f"xb{t}")
                      for t in range(2)]
            for t in range(2):
                nc.vector.tensor_scalar_mul(X_ping[t][:], id2w_sb[:],
                                            1.0 / 256.0)
            cur, nxt = X_ping, X_pong
            for it in range(NS_ITERS):
                for t in range(2):
                    P_ps = ps_ns.tile([128, 128], f32, tag="nsp")
                    nc.tensor.matmul(P_ps[:], W_bd[t][:], cur[t][:])
                    G_sb = nspool.tile([128, 128], f32, tag="nsg")
                    nc.vector.tensor_sub(G_sb[:], id2w_sb[:], P_ps[:])
                    Xp_ps = ps_ns.tile([128, 128], f32, tag="nsp")
                    nc.tensor.matmul(Xp_ps[:], cur[t][:], G_sb[:])
                    nc.vector.tensor_copy(nxt[t][:], Xp_ps[:])
                cur, nxt = nxt, cur
            M_bd = cur  # fp32 blockdiag inverse per pair

            # ---- stage 3: streaming projections + Phi + Z ----------------
            phiQ_sb = big.tile([128, 2, N], bf)
            Z_ps = ps_z.tile([128, 258], f32, tag="zacc")  # persistent bank

            def emit_sq(s, x8ts):
                nsl = slice(s * SLAB, (s + 1) * SLAB)
                for t in range(2):
                    SQ_ps = ps_big.tile([128, SLAB], f32, tag="psb",
                                        name=f"sqps{s}_{t}")
                    for c2 in range(4):
                        nc.tensor.matmul(
                            SQ_ps[:], aq8_sb[:, 2 * c2:2 * c2 + 2, t, :],
                            x8ts[:, 2 * c2:2 * c2 + 2, :], perf_mode=DR,
                            start=(c2 == 0), stop=(c2 == 3))
                    nc.scalar.activation(phiQ_sb[:, t, nsl], SQ_ps[:], Exp,
                                         scale=sc / 2048.0)

            xts_keep = {}
            for s in range(NSLAB):
                sp = s + XTS_PREF
                if sp < NSLAB:
                    xpf = slabs.tile([128, 8, SLAB], bf, tag="xts",
                                     bufs=XTS_BUFS, name=f"xts{sp}")
                    nc.sync.dma_start(out=xpf[:],
                                      in_=xT_r[:, :, sp * SLAB:(sp + 1) * SLAB])
                    xts_pref[sp] = xpf
                    x8pf = slabs.tile([128, 8, SLAB], f8e4, tag="x8ts",
                                      bufs=XTS_BUFS, name=f"x8ts{sp}")
                    nc.scalar.dma_start(
                        out=x8pf[:],
                        in_=x8_r[:, :, sp * SLAB:(sp + 1) * SLAB])
                    x8_pref[sp] = x8pf
                xts = xts_pref.pop(s)
                x8ts = x8_pref.pop(s)

                # K^T and squares
                KT = slabs.tile([128, 2, SLAB], bf, tag="kt", bufs=3)
                sqKT = slabs.tile([128, 2, SLAB], bf, tag="sqkt", bufs=3)
                for co in range(2):
                    K_ps = ps_big.tile([128, SLAB], f32, tag="psb")
                    for c2 in range(4):
                        nc.tensor.matmul(
                            K_ps[:],
                            wk8_sb[:, 2 * c2:2 * c2 + 2,
                                   co * 128:(co + 1) * 128],
                            x8ts[:, 2 * c2:2 * c2 + 2, :], perf_mode=DR,
                            start=(c2 == 0), stop=(c2 == 3))
                    nc.vector.tensor_scalar_mul(KT[:, co, :], K_ps[:],
                                                1.0 / 16384.0)
                    nc.scalar.activation(sqKT[:, co, :], K_ps[:], Square,
                                         scale=1.0 / 16384.0)

                # phiQ (folded Q projection); slabs 14/15 deferred to
                # stage 4 to fill the PE during the serial solve section
                if s < NSLAB - 2:
                    emit_sq(s, x8ts)
                else:
                    xts_keep[s] = x8ts

                # V (N-land), half-slab psums; Vb laid out per (chunk,
                # pair) as [V(128) | ones(1)].
                Vb = slabs.tile([128, NCH, 2 * 129], bf, tag="vb", bufs=3)
                nc.vector.memset(Vb[:, :, 128:129], 1.0)
                nc.vector.memset(Vb[:, :, 257:258], 1.0)
                for hf in range(2):
                    V_ps = ps_v.tile([128, 2 * C], f32, tag="psv")
                    for c in (2 * hf, 2 * hf + 1):
                        for ci in range(8):
                            nc.tensor.matmul(
                                V_ps[:, (c % 2) * C:(c % 2 + 1) * C],
                                xts[:, ci, c * 128:(c + 1) * 128],
                                wv_sb[:, ci, :], start=(ci == 0),
                                stop=(ci == 7))
                    for c in (2 * hf, 2 * hf + 1):
                        for t in range(2):
                            nc.vector.tensor_copy(
                                Vb[:, c, 129 * t:129 * t + 128],
                                V_ps[:, (c % 2) * C + 128 * t:
                                     (c % 2) * C + 128 * t + 128])

                # E_K (N-land) -> Phi_K', half-slab psums
                phiK = slabs.tile([128, NCH, C], bf, tag="phik", bufs=3)
                for hf in range(2):
                    E_ps = ps_e.tile([128, 2 * C], f32, tag="pse")
                    for c in (2 * hf, 2 * hf + 1):
                        for t in range(2):
                            cs = slice((c % 2) * C + 128 * t,
                                       (c % 2) * C + 128 * t + 128)
                            nc.tensor.matmul(E_ps[:, cs],
                                             KT[:, t, c * 128:(c + 1) * 128],
                                             bdl[:, t, :], start=True,
                                             stop=False)
                            nc.tensor.matmul(E_ps[:, cs],
                                             sqKT[:, t, c * 128:(c + 1) * 128],
                                             sqsel_sb[:], start=False,
                                             stop=True)
                    nc.scalar.activation(phiK[:, 2 * hf:2 * hf + 2, :],
                                         E_ps[:], Exp, scale=sc)

                # Z accumulated across ALL slabs in the persistent PSUM
                # bank: exactly one start (s==0 first mm) / stop (last).
                for c in range(NCH):
                    for t in range(2):
                        zc = 129 * t
                        nc.tensor.matmul(
                            Z_ps[:, zc:zc + 129],
                            phiK[:, c, 128 * t:128 * t + 128],
                            Vb[:, c, 129 * t:129 * t + 129],
                            start=(s == 0 and c == 0 and t == 0),
                            stop=(s == NSLAB - 1 and c == NCH - 1 and t == 1),
                            skip_group_check=True)

            # ---- stage 4: solve application (transposed), Wo folding -----
            # Zp pack from the persistent Z bank: cols [65t:65t+64] rows
            # 0:64 <- Z0 diag-block, rows 64:128 <- Z1 diag-block; col
            # 65t+64 <- s' (both halves valid).
            Zp_sb = singles.tile([128, 130], f32)
            for t in range(2):
                nc.vector.tensor_copy(Zp_sb[0:64, 65 * t:65 * t + 64],
                                      Z_ps[0:64, 129 * t:129 * t + 64])
                nc.vector.tensor_copy(Zp_sb[64:128, 65 * t:65 * t + 64],
                                      Z_ps[64:128, 129 * t + 64:129 * t + 128])
                nc.vector.tensor_copy(Zp_sb[:, 65 * t + 64:65 * t + 65],
                                      Z_ps[:, 129 * t + 128:129 * t + 129])

            # YT_pair = Zp_slice^T @ M_bd  (65 part: 64 chan + sY row? no —
            # rows = 64 chan cols of Zp slice + the s' col -> row 64 unused
            # here; Y^T blocks live in rows 0:64, cols [land0|land1]).
            YT_ps = []
            for t in range(2):
                yt = ps_big.tile([128, 128], f32, tag="psb", name=f"ytps{t}")
                nc.tensor.matmul(yt[0:65, :],
                                 Zp_sb[:, 65 * t:65 * t + 65],
                                 M_bd[t][:])
                YT_ps.append(yt)

            # sY = A @ s' (orig orientation, partition-dim vector per pair)
            sY_ps = ps_ns.tile([128, 2], f32, tag="nsp", name="syps")
            for t in range(2):
                nc.tensor.matmul(
                    sY_ps[:, t:t + 1], M_bd[t][:],
                    Zp_sb[:, 65 * t + 64:65 * t + 65],
                    skip_group_check=True)

            # deferred slab-14 phiQ fills the PE while the DVE does the
            # stage-4 packing / selYE construction
            emit_sq(NSLAB - 2, xts_keep[NSLAB - 2])

            # Y_bdT (chan-part, land-cols) blockdiag, bf16
            Y_bdT = singles.tile([128, 2, 128], bf)
            nc.vector.memset(Y_bdT[:], 0.0)
            for t in range(2):
                nc.vector.tensor_copy(Y_bdT[0:64, t, 0:64],
                                      YT_ps[t][0:64, 0:64])
                nc.vector.tensor_copy(Y_bdT[64:128, t, 64:128],
                                      YT_ps[t][0:64, 64:128])

            # Ypack[t] = Y_bdT[t]^T-contract @ wo_pair  (land-part, embed)
            Ypack = singles.tile([128, 2, EMBED_DIM], bf)
            for t in range(2):
                for eh in range(2):
                    yp_ps = ps_ns.tile([128, 512], f32, tag="nsp",
                                       name=f"ypps{t}_{eh}")
                    nc.tensor.matmul(yp_ps[:], Y_bdT[:, t, :],
                                     wo_sb[:, t, eh * 512:eh * 512 + 512])
                    nc.vector.tensor_copy(
                        Ypack[:, t, eh * 512:eh * 512 + 512], yp_ps[:])

            emit_sq(NSLAB - 1, xts_keep[NSLAB - 1])

            # selYE[t]: blockdiag broadcast of sY per 64-row head block
            selYE = singles.tile([128, 2, 128], bf)
            nc.vector.memset(selYE[:], 0.0)
            for t in range(2):
                for hh in range(2):
                    r = slice(64 * hh, 64 * hh + 64)
                    nc.vector.tensor_copy(
                        selYE[r, t, 64 * hh:64 * hh + 64],
                        sY_ps[r, t:t + 1].broadcast_to([64, 64]))

            # ---- stage 5: norm -> 1/norm -> phiQn -> transposed O-proj ---
            for s in range(NSLAB):
                nsl = slice(s * SLAB, (s + 1) * SLAB)
                rnE = slabs.tile([128, 2, SLAB], f32, tag="rne", bufs=3)
                phiQn = slabs.tile([128, 2, SLAB], bf, tag="pqn", bufs=3)
                for t in range(2):
                    n_ps = ps_big.tile([128, SLAB], f32, tag="psb",
                                       name=f"nps{s}_{t}")
                    nc.tensor.matmul(n_ps[:], selYE[:, t, :],
                                     phiQ_sb[:, t, nsl])
                    nc.vector.reciprocal_approx_fast(out=rnE[:, t, :],
                                                     in_=n_ps[:])
                    nc.vector.tensor_mul(phiQn[:, t, :], phiQ_sb[:, t, nsl],
                                         rnE[:, t, :])

                oout = slabs.tile([128, 8, SLAB], bf, tag="oout", bufs=3)
                for e in range(8):
                    pool = ps_v if e % 2 == 0 else ps_e
                    O_ps = pool.tile([128, SLAB], f32,
                                     tag=("psv" if e % 2 == 0 else "pse"),
                                     name=f"ops{s}_{e}")
                    for t in range(2):
                        nc.tensor.matmul(
                            O_ps[:], Ypack[:, t, e * 128:(e + 1) * 128],
                            phiQn[:, t, :], start=(t == 0), stop=(t == 1))
                    if e < 2:
                        nc.vector.tensor_copy(oout[:, e, :], O_ps[:])
                    else:
                        nc.scalar.activation(oout[:, e, :], O_ps[:], Copy)
                for q in range(4):
                    nc.sync.dma_start(
                        out=out_rT[:, 2 * q:2 * q + 2, nsl],
                        in_=oout[:, 2 * q:2 * q + 2, :])
                if DEBUG_DUMP and s == 0:
                    rne_dbg = nc.dram_tensor("rnedbg", [128, 2, SLAB], f32,
                                             kind="ExternalOutput")
                    pqn_dbg = nc.dram_tensor("pqndbg", [128, 2, SLAB], bf,
                                             kind="ExternalOutput")
                    nc.sync.dma_start(out=rne_dbg.ap(), in_=rnE[:])
                    nc.sync.dma_start(out=pqn_dbg.ap(), in_=phiQn[:])

            if DEBUG_DUMP:
                z_dbg = nc.dram_tensor("zdbg", [128, 258], f32,
                                       kind="ExternalOutput")
                zc_sb = singles.tile([128, 258], f32)
                nc.vector.tensor_copy(zc_sb[:], Z_ps[:])
                nc.sync.dma_start(out=z_dbg.ap(), in_=zc_sb[:])
                zp_dbg = nc.dram_tensor("zpdbg", [128, 130], f32,
                                        kind="ExternalOutput")
                nc.sync.dma_start(out=zp_dbg.ap(), in_=Zp_sb[:])
                yt_dbg = nc.dram_tensor("ytdbg", [128, 2, 128], f32,
                                        kind="ExternalOutput")
                yt_sb = singles.tile([128, 2, 128], f32)
                for t in range(2):
                    nc.vector.tensor_copy(yt_sb[:, t, :], YT_ps[t][:])
                nc.sync.dma_start(out=yt_dbg.ap(), in_=yt_sb[:])
                sy_dbg = nc.dram_tensor("sydbg", [128, 2], f32,
                                        kind="ExternalOutput")
                sy_sb = singles.tile([128, 2], f32)
                nc.vector.tensor_copy(sy_sb[:], sY_ps[:])
                nc.sync.dma_start(out=sy_dbg.ap(), in_=sy_sb[:])
                sel_dbg = nc.dram_tensor("seldbg", [128, 2, 128], bf,
                                         kind="ExternalOutput")
                nc.sync.dma_start(out=sel_dbg.ap(), in_=selYE[:])
                yp_dbg = nc.dram_tensor("ypdbg", [128, 2, 1024], bf,
                                        kind="ExternalOutput")
                nc.sync.dma_start(out=yp_dbg.ap(), in_=Ypack[:])
                ybdt_dbg = nc.dram_tensor("ybdtdbg", [128, 2, 128], bf,
                                          kind="ExternalOutput")
                nc.sync.dma_start(out=ybdt_dbg.ap(), in_=Y_bdT[:])
                phiq_dbg = nc.dram_tensor("phiqdbg", [128, 2, SLAB], bf,
                                          kind="ExternalOutput")
                nc.sync.dma_start(out=phiq_dbg.ap(),
                                  in_=phiQ_sb[:, :, 0:SLAB])
                mbd_dbg = nc.dram_tensor("mbddbg", [128, 2, 128], f32,
                                         kind="ExternalOutput")
                mb_sb = singles.tile([128, 2, 128], f32)
                for t in range(2):
                    nc.vector.tensor_copy(mb_sb[:, t, :], M_bd[t][:])
                nc.sync.dma_start(out=mbd_dbg.ap(), in_=mb_sb[:])
                xw_dbg = nc.dram_tensor("xwdbg", [128, 2, 128], f32,
                                        kind="ExternalOutput")
                xw_sb = singles.tile([128, 2, 128], f32)
                for t in range(2):
                    nc.vector.tensor_copy(xw_sb[:, t, :], cur[t][:])
                nc.sync.dma_start(out=xw_dbg.ap(), in_=xw_sb[:])
                wbd_dbg = nc.dram_tensor("wbddbg", [128, 2, 128], f32,
                                         kind="ExternalOutput")
                wb_sb = singles.tile([128, 2, 128], f32)
                for t in range(2):
                    nc.vector.tensor_copy(wb_sb[:, t, :], W_bd[t][:])
                nc.sync.dma_start(out=wbd_dbg.ap(), in_=wb_sb[:])
    nc.compile()
    return nc


_NC_CACHE = None


def _get_nc(tau):
    global _NC_CACHE
    if _NC_CACHE is None:
        _NC_CACHE = _build_bass(tau)
    return _NC_CACHE


# ---------------------------------------------------------------------------
# host marshalling
# ---------------------------------------------------------------------------

def _consts():
    sqsel = np.zeros((128, 128), np.float32)
    sqsel[0:64, 0:64] = -0.5
    sqsel[64:128, 64:128] = -0.5
    id2w = 2.0 * np.eye(128, dtype=np.float32)
    return (sqsel.astype(BF16), id2w)


F8 = ml_dtypes.float8_e4m3fn
X8_SCALE = 32.0
WK8_SCALE = 512.0
AQ8_SCALE = 64.0


def _f8(a, s):
    return np.clip(np.asarray(a, np.float32) * s, -240.0, 240.0).astype(F8)


def _kernel_device(query, Wq, Wk, Wv, Wo, bo, tau, idx):
    global LAST_RESULTS
    from concourse.bass_utils import run_bass_kernel_spmd

    nc = _get_nc(tau)
    b, n, _ = query.shape

    sqsel, id2w = _consts()
    WkT = np.ascontiguousarray(Wk.T).astype(BF16)
    WvT = np.ascontiguousarray(Wv.T).astype(BF16)
    WoT = np.ascontiguousarray(Wo.T).astype(BF16)
    Wq_bf = Wq.astype(BF16)

    in_maps = []
    for bi in range(b):
        xT = np.ascontiguousarray(query[bi].T).astype(BF16)
        x8 = _f8(xT.astype(np.float32), X8_SCALE)
        xlT = np.ascontiguousarray(query[bi][idx].T).astype(BF16)
        xlT_f = xlT.astype(np.float32)
        for g in range(GROUPS):
            sl = slice(g * C, (g + 1) * C)
            wk_bf = np.ascontiguousarray(WkT[:, sl])
            # host A_q = blockdiag(L) @ Wq_pair, fp8-packed for DoubleRow
            LT = (wk_bf.astype(np.float32).T @ xlT_f).astype(BF16).astype(
                np.float32)                                   # (C, NL)
            aq8 = np.zeros((128, 8, 2, 128), F8)
            for t in range(2):
                bdl = np.zeros((128, 128), np.float32)
                bdl[0:64, 0:64] = LT[128 * t:128 * t + 64, :]
                bdl[64:128, 64:128] = LT[128 * t + 64:128 * t + 128, :]
                bdl = bdl.astype(BF16).astype(np.float32)
                wq_pair = Wq_bf[sl][128 * t:128 * t + 128, :].astype(
                    np.float32)                               # (128, E)
                A_qT = wq_pair.T @ bdl                        # (E, 128)
                aq8[:, :, t, :] = _f8(
                    A_qT.reshape(8, 128, 128).transpose(1, 0, 2), AQ8_SCALE)
            in_maps.append({
                "xT": xT,
                "x8": x8,
                "xlT": xlT,
                "wkT": wk_bf,
                "wk8": _f8(wk_bf.astype(np.float32), WK8_SCALE),
                "aq8": aq8,
                "wvT": np.ascontiguousarray(WvT[:, sl]),
                "woT": np.ascontiguousarray(WoT[sl, :]),
                "sqsel": sqsel,
                "id2w": id2w,
            })

    res = run_bass_kernel_spmd(nc, in_maps, core_ids=list(range(N_CORES)))
    LAST_RESULTS = res

    out = np.zeros((b, n, EMBED_DIM), np.float32)
    for bi in range(b):
        for g in range(GROUPS):
            out[bi] += res.results[bi * GROUPS + g]["opart"].astype(
                np.float32).T
    out += bo
    return out


def _kernel_numpy(query, Wq, bq, Wk, bk, Wv, bv, Wo, bo, tau, idx):
    """Reference-faithful fallback (nonzero biases etc.)."""
    b, n, _ = query.shape
    out = np.zeros((b, n, EMBED_DIM), np.float32)
    for bi in range(b):
        x = query[bi]
        Q = (x @ Wq.T + bq).reshape(n, NUM_HEADS, HEAD_DIM).transpose(1, 0, 2)
        K = (x @ Wk.T + bk).reshape(n, NUM_HEADS, HEAD_DIM).transpose(1, 0, 2)
        V = (x @ Wv.T + bv).reshape(n, NUM_HEADS, HEAD_DIM).transpose(1, 0, 2)
        L = K[:, idx, :]
        def sqd(X, Lm):
            Xn = np.sum(X * X, -1, keepdims=True)
            Ln = np.sum(Lm * Lm, -1, keepdims=True)
            return np.maximum(Xn + np.swapaxes(Ln, -2, -1)
                              - 2.0 * np.einsum("hnd,hkd->hnk", X, Lm), 0.0)
        PhiQ = np.exp(-sqd(Q, L) / tau)
        PhiK = np.exp(-sqd(K, L) / tau)
        Wk_ = np.exp(-sqd(L, L) / tau) + 1e-6 * np.eye(NL, dtype=np.float32)
        Z = np.einsum("hnk,hnd->hkd", PhiK, V)
        Y = np.linalg.solve(Wk_, Z)
        ou = np.einsum("hnk,hkd->hnd", PhiQ, Y)
        sY = np.linalg.solve(Wk_, PhiK.sum(1)[..., None])
        nrm = np.maximum(np.einsum("hnk,hko->hno", PhiQ, sY), 1e-10)
        H = (ou / nrm).transpose(1, 0, 2).reshape(n, EMBED_DIM)
        out[bi] = H @ Wo.T
    return out + bo


def kernel(query, Wq, bq, Wk, bk, Wv, bv, Wo, bo, temperature, landmark_idx):
    query = np.asarray(query, dtype=np.float32)
    Wq, Wk, Wv, Wo = (np.asarray(w, np.float32) for w in (Wq, Wk, Wv, Wo))
    bq, bk, bv, bo = (np.asarray(x, np.float32) for x in (bq, bk, bv, bo))
    tau = float(np.asarray(temperature))
    idx = np.asarray(landmark_idx).astype(np.int64)

    if (query.shape != (2, N, EMBED_DIM) or idx.shape != (NL,)
            or np.any(bq) or np.any(bk) or np.any(bv)):
        return _kernel_numpy(query, Wq, bq, Wk, bk, Wv, bv, Wo, bo, tau, idx)
    return _kernel_device(query, Wq, Wk, Wv, Wo, bo, tau, idx).astype(
        np.float32, copy=False)
